# revision 1
# baseline (speedup 1.0000x reference)
"""Trainium2 Bass kernel for GCE-GNN LocalAggregator (gnn_message_passing).

Computes, for each batch b:
    h = embedding[inputs]                            # [N, D] gather
    e_k = leakyrelu((h * a_k) @ h.T, 0.2)            # k = 0..3
    alpha = softmax(where(adj == k+1, e_k, -inf))    # edge-type select
    out = alpha @ h

Sharding: data-parallel over batch B=512 across 8 cores (64 batches/core).
The embedding table (22MB) is replicated in each core's DRAM; only looked-up
rows are read, via indirect (gather) DMA.

Per-core device pipeline (Bc=64 local batches; "pair" = 2 batches sharing
the 128 partitions as (u, i) with u in {0,1}):
  1. 64 indirect DMAs gather h rows: h_b = [64 i, 128 d] per batch.
  2. PE transposes h_b -> hT (d on partitions); ACT evacuates to bf16.
  3. S_all[k] = hT * a_k (DVE bulk, bf16) -> rhs of the attention matmul.
  4. e matmuls (bf16 in, f32 PSUM): e[b; i, (k,j)] = sum_d h[i,d] a_k[d] h[j,d],
     two matmuls per pair writing the two partition halves of one PSUM tile.
  5. ACT evacuates raw e to SBUF; DVE selects per (i,j) the e_k with
     adj==k+1 (copy_predicated over a -1e9 background).
  6. exp(leakyrelu(x)) == max(exp(x), exp(0.2x)): two ACT Exp passes (the
     0.2 rides the activation's scale) + a GPSIMD max. Masked entries give
     exp(-1e9 * {1, 0.2}) = 0, so softmax needs no -inf handling and rows
     sum correctly.
  7. Row sums (DVE reduce) + reciprocal give softmax scales r.
  8. PE transposes x per pair -> xT [j, (u,i)]; out-matmuls per batch write
     the two halves of a pair PSUM tile; ACT evacuates with scale=r
     (fusing the softmax divide into the copy); one DMA per pair to DRAM.
"""

import os
import sys

import numpy as np

for _p in ("/opt/trn_rl_repo",):
    if _p not in sys.path and os.path.isdir(_p):
        sys.path.insert(0, _p)

import concourse.bass as bass
import concourse.bacc as bacc
import concourse.tile as tile
from concourse import mybir
from concourse.bass_utils import run_bass_kernel_spmd
from concourse.masks import make_identity

B, N, D, V = 512, 64, 128, 43098
NCORES = 8
BC = B // NCORES          # 64 local batches per core
NPAIR = BC // 2           # 32 pairs
ALPHA = 0.2
NEG_BIG = -1.0e9          # exp(NEG_BIG) == 0 in fp32; stands in for -9e15

FP32 = mybir.dt.float32
BF16 = mybir.dt.bfloat16
INT32 = mybir.dt.int32
AF = mybir.ActivationFunctionType
OP = mybir.AluOpType


def build_nc():
    nc = bacc.Bacc("TRN2", target_bir_lowering=False, debug=False)

    idx_d = nc.dram_tensor("idx", [N, BC], INT32, kind="ExternalInput")
    msk_d = nc.dram_tensor("mask4", [128, 4 * NPAIR * N], mybir.dt.uint8, kind="ExternalInput")
    emb_d = nc.dram_tensor("emb", [V, D], FP32, kind="ExternalInput")
    attn_d = nc.dram_tensor("attnT", [D, 4], FP32, kind="ExternalInput")
    out_d = nc.dram_tensor("out", [BC, N, D], FP32, kind="ExternalOutput")

    with tile.TileContext(nc) as tc:
        with (
            tc.tile_pool(name="singles", bufs=1) as singles,
            tc.tile_pool(name="big", bufs=1) as big,
            tc.tile_pool(name="work", bufs=3) as work,
            tc.tile_pool(name="outp", bufs=3) as outp,
            tc.tile_pool(name="ps_h", bufs=2, space="PSUM") as ps_h,
            tc.tile_pool(name="ps_e", bufs=2, space="PSUM") as ps_e,
            tc.tile_pool(name="ps_x", bufs=2, space="PSUM") as ps_x,
            tc.tile_pool(name="ps_o", bufs=2, space="PSUM") as ps_o,
        ):
            # ---- constants / inputs ----
            idx_sb = singles.tile([N, BC], INT32)
            nc.sync.dma_start(out=idx_sb[:, :], in_=idx_d[:, :])
            attn_sb = singles.tile([128, 4], FP32)
            nc.sync.dma_start(out=attn_sb[:, :], in_=attn_d[:, :])
            ident = singles.tile([128, 128], FP32)
            make_identity(nc, ident[:, :])
            # host-precomputed edge-type masks: [p=(u,i), k, (g, j)] uint8
            msk_sb = big.tile([128, 4, NPAIR, N], mybir.dt.uint8, tag="msk")
            nc.sync.dma_start(
                out=msk_sb[:, :, :, :],
                in_=msk_d.ap().rearrange("p (k g j) -> p k g j", k=4, j=N),
            )

            # ---- gather h per batch: h_b[i, d] ----
            h_nat = []
            for b in range(BC):
                h = big.tile([N, D], FP32, tag=f"h{b}")
                nc.gpsimd.indirect_dma_start(
                    out=h[:, :],
                    out_offset=None,
                    in_=emb_d[:, :],
                    in_offset=bass.IndirectOffsetOnAxis(
                        ap=idx_sb[:, b : b + 1], axis=0
                    ),
                )
                h_nat.append(h)

            # hT_all[p=d, b, i] : bf16
            hT_all = big.tile([128, BC, N], BF16, tag="hT")
            # S_all[p=d, k, b, i] : a_k-scaled hT, bf16
            S_all = big.tile([128, 4, BC, N], BF16, tag="S")
            # e_all[p=(u,i), k, g, j] : raw attention logits, f32
            e_all = big.tile([128, 4, NPAIR, N], FP32, tag="e")
            # select/exp buffers [p=(u,i), g, j]
            alpha = big.tile([128, NPAIR, N], FP32, tag="alpha")
            xe1 = big.tile([128, NPAIR, N], FP32, tag="xe1")
            xe2 = big.tile([128, NPAIR, N], FP32, tag="xe2")
            x_all = big.tile([128, NPAIR, N], FP32, tag="x")
            s_sum = singles.tile([128, NPAIR], FP32)
            r_inv = singles.tile([128, NPAIR], FP32)

            # ---- transpose h per batch, evac to bf16 ----
            for b in range(BC):
                hT_ps = ps_h.tile([128, N], FP32, tag="hT_ps")
                nc.tensor.transpose(
                    out=hT_ps[:, :],
                    in_=h_nat[b][:, :],
                    identity=ident[:N, :N],
                )
                nc.scalar.copy(out=hT_all[:, b, :], in_=hT_ps[:, :])

            # ---- S_all = hT * a_k (bulk, 4 ops) ----
            for k in range(4):
                nc.vector.tensor_scalar_mul(
                    out=S_all[:, k, :, :].rearrange("p b i -> p (b i)"),
                    in0=hT_all[:, :, :].rearrange("p b i -> p (b i)"),
                    scalar1=attn_sb[:, k : k + 1],
                )

            # ---- e matmuls + evac ----
            for g in range(NPAIR):
                e_ps = ps_e.tile([128, 4, N], FP32, tag="e_ps")
                for u in range(2):
                    b = 2 * g + u
                    nc.tensor.matmul(
                        out=e_ps[u * 64 : (u + 1) * 64, :, :],
                        lhsT=hT_all[:, b, :],
                        rhs=S_all[:, :, b, :],
                        start=True,
                        stop=True,
                    )
                nc.scalar.copy(out=e_all[:, :, g, :], in_=e_ps[:, :, :])

            # ---- select e_k by adj (background NEG_BIG) ----
            nc.gpsimd.memset(alpha[:, :, :], NEG_BIG)
            for k in range(4):
                nc.vector.copy_predicated(
                    out=alpha[:, :, :],
                    mask=msk_sb[:, k, :, :],
                    data=e_all[:, k, :, :],
                )

            # ---- x = exp(leakyrelu(alpha)) = max(exp(alpha), exp(0.2*alpha)) ----
            flat = "p g j -> p (g j)"
            nc.scalar.activation(
                out=xe1[:, :, :].rearrange(flat),
                in_=alpha[:, :, :].rearrange(flat),
                func=AF.Exp,
            )
            nc.scalar.activation(
                out=xe2[:, :, :].rearrange(flat),
                in_=alpha[:, :, :].rearrange(flat),
                func=AF.Exp,
                scale=ALPHA,
            )
            nc.vector.tensor_tensor(
                out=x_all[:, :, :],
                in0=xe1[:, :, :],
                in1=xe2[:, :, :],
                op=OP.max,
            )

            # ---- softmax denominators ----
            nc.vector.reduce_sum(
                out=s_sum[:, :], in_=x_all[:, :, :], axis=mybir.AxisListType.X
            )
            nc.vector.reciprocal(out=r_inv[:, :], in_=s_sum[:, :])

            # ---- transpose x per pair; out matmuls; scaled evac; DMA ----
            for g in range(NPAIR):
                xT_ps = ps_x.tile([N, 128], FP32, tag="xT_ps")
                nc.tensor.transpose(
                    out=xT_ps[:, :],
                    in_=x_all[:, g, :],
                    identity=ident[:, :],
                )
                xT_sb = work.tile([N, 128], FP32, tag="xT_sb")
                nc.vector.tensor_copy(out=xT_sb[:, :], in_=xT_ps[:, :])
                o_ps = ps_o.tile([128, D], FP32, tag="o_ps")
                for u in range(2):
                    b = 2 * g + u
                    nc.tensor.matmul(
                        out=o_ps[u * 64 : (u + 1) * 64, :],
                        lhsT=xT_sb[:, u * 64 : (u + 1) * 64],
                        rhs=h_nat[b][:, :],
                        start=True,
                        stop=True,
                    )
                o_sb = outp.tile([128, D], FP32, tag="o_sb")
                nc.scalar.activation(
                    out=o_sb[:, :],
                    in_=o_ps[:, :],
                    func=AF.Copy,
                    scale=r_inv[:, g : g + 1],
                )
                nc.sync.dma_start(
                    out=out_d.ap().rearrange("b i d -> (b i) d")[
                        128 * g : 128 * (g + 1), :
                    ],
                    in_=o_sb[:, :],
                )
    nc.compile()
    return nc


_CACHE = {}


def _compiled():
    if "nc" not in _CACHE:
        _CACHE["nc"] = build_nc()
    return _CACHE["nc"]


def _shard_inputs(inputs, adj, embedding, attn_a):
    inputs = np.asarray(inputs)
    adj = np.asarray(adj)
    emb = np.ascontiguousarray(np.asarray(embedding, dtype=np.float32))
    attnT = np.ascontiguousarray(np.asarray(attn_a, dtype=np.float32).T)  # [D, 4]
    in_maps = []
    for c in range(NCORES):
        sl = slice(c * BC, (c + 1) * BC)
        # idx[p, b] = inputs[c*BC + b, p]
        idx_t = np.ascontiguousarray(inputs[sl].astype(np.int32).T)  # [N, BC]
        adj_c = adj[sl].astype(np.int32)  # [BC, N, N]
        # [p=(u,i), (g, j)]
        adj_big = adj_c.reshape(NPAIR, 2, N, N).transpose(1, 2, 0, 3).reshape(
            128, NPAIR * N
        )
        # one-hot edge-type masks, [p, k, (g, j)] packed to [128, 4*NPAIR*N]
        mask4 = np.ascontiguousarray(
            np.stack([(adj_big == k + 1) for k in range(4)], axis=1)
            .astype(np.uint8)
            .reshape(128, 4 * NPAIR * N)
        )
        in_maps.append(dict(idx=idx_t, mask4=mask4, emb=emb, attnT=attnT))
    return in_maps


def kernel(inputs, adj, mask_item, item, embedding, attn_a):
    in_maps = _shard_inputs(inputs, adj, embedding, attn_a)
    res = run_bass_kernel_spmd(
        _compiled(), in_maps, core_ids=list(range(NCORES))
    ).results
    out = np.concatenate([np.asarray(res[c]["out"]) for c in range(NCORES)], axis=0)
    return out.astype(np.float32)



# revision 6
# speedup vs baseline: 1.7418x; 1.7418x over previous
"""Trainium2 Bass kernel for GCE-GNN LocalAggregator (gnn_message_passing).

Computes, for each batch b:
    h = embedding[inputs]                            # [N, D] gather
    e_k = leakyrelu((h * a_k) @ h.T, 0.2)            # k = 0..3
    alpha = softmax(where(adj == k+1, e_k, -inf))    # edge-type select
    out = alpha @ h

Sharding: data-parallel over batch B=512 across 8 cores (64 batches/core).
The embedding table is staged in bf16 (11MB) per core; only looked-up rows
are read from HBM.

Key structural ideas vs the naive mapping:
  * ONE bulk indirect DMA (per half) gathers all 4096 rows into the pair
    layout h_pair[(u,i), g, d] - the offset AP carries 32 indices per
    partition, so the SWDGE descriptor generation is paid once (~1us) rather
    than 64 times.
  * hT (d on partitions) comes from 32 PE transposes of whole PAIRS
    ([128,128] each), not 64 per-batch transposes.
  * e_k is SYMMETRIC in (i,j) (e_k[i,j] = sum_d h_i h_j a_k). The transposed
    attention matrix xT (the out-matmul lhsT) is obtained by selecting from
    the SAME e buffer with host-transposed masks - no transpose of x at all.
    Row sums for the softmax come from the untransposed selection.
  * exp(leakyrelu(x)) == max(exp(x), exp(0.2x)); masked entries are -1e9 so
    exp gives exact 0 and softmax needs no -inf handling.
  * bf16 everywhere off-PSUM: half the DMA bytes, 2x DVE throughput, fast
    bf16 matmuls (tolerance is 2e-2; bf16 end-to-end lands ~2e-3).
  * Work is chunked in groups of 8 pairs so select/exp/out-matmul of chunk c
    overlap the e-matmuls of chunk c+1 across engines.
"""

import os
import sys

import numpy as np

for _p in ("/opt/trn_rl_repo",):
    if _p not in sys.path and os.path.isdir(_p):
        sys.path.insert(0, _p)

import ml_dtypes

import concourse.bass as bass
import concourse.bacc as bacc
import concourse.tile as tile
from concourse import mybir
from concourse.bass_utils import run_bass_kernel_spmd
from concourse.masks import make_identity

B, N, D, V = 512, 64, 128, 43098
NCORES = 8
BC = B // NCORES          # 64 local batches per core
NPAIR = BC // 2           # 32 pairs
ALPHA = 0.2
NEG_BIG = -1.0e9          # exp(NEG_BIG) == 0 in fp32; stands in for -9e15
CH = 8                    # pairs per pipeline chunk
NCH = NPAIR // CH

FP32 = mybir.dt.float32
BF16 = mybir.dt.bfloat16
I32 = mybir.dt.int32
U8 = mybir.dt.uint8
AF = mybir.ActivationFunctionType
OP = mybir.AluOpType
AX = mybir.AxisListType

BF = ml_dtypes.bfloat16


def build_nc():
    nc = bacc.Bacc("TRN2", target_bir_lowering=False, debug=False)

    emb_d = nc.dram_tensor("emb", [V, D], BF16, kind="ExternalInput")
    idx_d = nc.dram_tensor("idx", [128, NPAIR], I32, kind="ExternalInput")
    mx_d = nc.dram_tensor("mx", [128, 4 * NPAIR * N], U8, kind="ExternalInput")
    mxT_d = nc.dram_tensor("mxT", [128, 4 * NPAIR * N], U8, kind="ExternalInput")
    attn_d = nc.dram_tensor("attnT", [D, 4], FP32, kind="ExternalInput")
    out_d = nc.dram_tensor("out", [BC, N, D], FP32, kind="ExternalOutput")

    with tile.TileContext(nc) as tc:
        with (
            tc.tile_pool(name="singles", bufs=1) as singles,
            tc.tile_pool(name="big", bufs=1) as big,
            tc.tile_pool(name="chnk", bufs=2) as chnk,
            tc.tile_pool(name="outp", bufs=3) as outp,
            tc.tile_pool(name="ps_t", bufs=2, space="PSUM") as ps_t,
            tc.tile_pool(name="ps_e", bufs=3, space="PSUM") as ps_e,
            tc.tile_pool(name="ps_o", bufs=3, space="PSUM") as ps_o,
        ):
            # ---- tiny inputs ----
            idx_sb = singles.tile([128, NPAIR], I32)
            nc.sync.dma_start(out=idx_sb[:, :], in_=idx_d[:, :])
            attn_sb = singles.tile([128, 4], FP32)
            nc.sync.dma_start(out=attn_sb[:, :], in_=attn_d[:, :])
            ident = singles.tile([128, 128], BF16)
            make_identity(nc, ident[:, :])

            # edge-type masks, [p, k, g, f] uint8 (f = j for mx, i for mxT)
            mx_sb = big.tile([128, 4, NPAIR, N], U8, tag="mx")
            nc.sync.dma_start(
                out=mx_sb[:, :, :, :],
                in_=mx_d.ap().rearrange("p (k g j) -> p k g j", k=4, j=N),
            )
            mxT_sb = big.tile([128, 4, NPAIR, N], U8, tag="mxT")
            nc.sync.dma_start(
                out=mxT_sb[:, :, :, :],
                in_=mxT_d.ap().rearrange("p (k g j) -> p k g j", k=4, j=N),
            )

            # h_pair[(u,i), g, d], hT[d, g, (u,i)], S[d, k, g, (u,i)]
            hp = big.tile([128, NPAIR, D], BF16, tag="hp")
            hT = big.tile([128, NPAIR, 128], BF16, tag="hT")
            S_all = big.tile([128, 4, NPAIR, 128], BF16, tag="S")

            # ---- gather one pair (128 rows) per indirect DMA; the HW DGE
            # supports one offset per partition per instruction ----
            GH = NPAIR // 2
            for h in range(2):
                gs = slice(h * GH, (h + 1) * GH)
                for g in range(h * GH, (h + 1) * GH):
                    nc.gpsimd.indirect_dma_start(
                        out=hp[:, g, :],
                        out_offset=None,
                        in_=emb_d[:, :],
                        in_offset=bass.IndirectOffsetOnAxis(
                            ap=idx_sb[:, g : g + 1], axis=0
                        ),
                    )
                # pair transposes h_pair[:,g,:] -> hT[:,g,:], evac to bf16
                for g in range(h * GH, (h + 1) * GH):
                    t_ps = ps_t.tile([128, 128], BF16, tag="t_ps")
                    nc.tensor.transpose(
                        out=t_ps[:, :], in_=hp[:, g, :], identity=ident[:, :]
                    )
                    if g % 2 == 0:
                        nc.scalar.copy(out=hT[:, g, :], in_=t_ps[:, :])
                    else:
                        nc.vector.tensor_copy(out=hT[:, g, :], in_=t_ps[:, :])
                # S = hT * a_k for this half's pairs
                for k in range(4):
                    nc.vector.tensor_scalar_mul(
                        out=S_all[:, k, gs, :].rearrange("p g q -> p (g q)"),
                        in0=hT[:, gs, :].rearrange("p g q -> p (g q)"),
                        scalar1=attn_sb[:, k : k + 1],
                    )

            flat = "p g j -> p (g j)"
            for c in range(NCH):
                g0 = c * CH
                # ---- chunk tiles ----
                e_c = chnk.tile([128, 4, CH, N], BF16, tag="e")
                ax = chnk.tile([128, CH, N], BF16, tag="ax")
                axT = chnk.tile([128, CH, N], BF16, tag="axT")
                xe1 = chnk.tile([128, CH, N], BF16, tag="xe1")
                xe2 = chnk.tile([128, CH, N], BF16, tag="xe2")
                xe3 = chnk.tile([128, CH, N], BF16, tag="xe3")
                xe4 = chnk.tile([128, CH, N], BF16, tag="xe4")
                xs = chnk.tile([128, CH, N], BF16, tag="xs")
                xT = chnk.tile([128, CH, N], BF16, tag="xT")
                ssum = chnk.tile([128, CH], FP32, tag="ssum")
                rinv = chnk.tile([128, CH], FP32, tag="rinv")

                nc.gpsimd.memset(ax[:, :, :], NEG_BIG)
                nc.gpsimd.memset(axT[:, :, :], NEG_BIG)

                # ---- e matmuls: e[b; i, (k,j)] = sum_d h[i,d] a_k[d] h[j,d] ----
                for gl in range(CH):
                    g = g0 + gl
                    e_ps = ps_e.tile([128, 4, N], FP32, tag="e_ps")
                    for u in range(2):
                        nc.tensor.matmul(
                            out=e_ps[u * 64 : (u + 1) * 64, :, :],
                            lhsT=hT[:, g, u * 64 : (u + 1) * 64],
                            rhs=S_all[:, :, g, u * 64 : (u + 1) * 64],
                            start=True,
                            stop=True,
                        )
                    if gl % 2 == 0:
                        nc.scalar.copy(out=e_c[:, :, gl, :], in_=e_ps[:, :, :])
                    else:
                        nc.vector.tensor_copy(out=e_c[:, :, gl, :], in_=e_ps[:, :, :])

                # ---- edge-type select; by symmetry of e_k the same buffer
                # read as [(v,j), k, g, i] gives the transposed logits ----
                for k in range(4):
                    nc.vector.copy_predicated(
                        out=ax[:, :, :],
                        mask=mx_sb[:, k, g0 : g0 + CH, :],
                        data=e_c[:, k, :, :],
                    )
                for k in range(4):
                    nc.vector.copy_predicated(
                        out=axT[:, :, :],
                        mask=mxT_sb[:, k, g0 : g0 + CH, :],
                        data=e_c[:, k, :, :],
                    )

                # ---- x = exp(leakyrelu(.)) = max(exp(.), exp(0.2 .)) ----
                nc.scalar.activation(
                    out=xe1[:, :, :].rearrange(flat),
                    in_=ax[:, :, :].rearrange(flat),
                    func=AF.Exp,
                )
                nc.scalar.activation(
                    out=xe2[:, :, :].rearrange(flat),
                    in_=ax[:, :, :].rearrange(flat),
                    func=AF.Exp,
                    scale=ALPHA,
                )
                nc.vector.tensor_tensor(
                    out=xs[:, :, :], in0=xe1[:, :, :], in1=xe2[:, :, :], op=OP.max
                )
                nc.scalar.activation(
                    out=xe3[:, :, :].rearrange(flat),
                    in_=axT[:, :, :].rearrange(flat),
                    func=AF.Exp,
                )
                nc.scalar.activation(
                    out=xe4[:, :, :].rearrange(flat),
                    in_=axT[:, :, :].rearrange(flat),
                    func=AF.Exp,
                    scale=ALPHA,
                )
                nc.vector.tensor_tensor(
                    out=xT[:, :, :], in0=xe3[:, :, :], in1=xe4[:, :, :], op=OP.max
                )

                # ---- softmax denominators ----
                nc.vector.reduce_sum(out=ssum[:, :], in_=xs[:, :, :], axis=AX.X)
                nc.vector.reciprocal(out=rinv[:, :], in_=ssum[:, :])

                # ---- out matmuls: out[i,d] = sum_j x[i,j] h[j,d], scaled ----
                for gl in range(CH):
                    g = g0 + gl
                    o_ps = ps_o.tile([128, D], FP32, tag="o_ps")
                    for u in range(2):
                        nc.tensor.matmul(
                            out=o_ps[u * 64 : (u + 1) * 64, :],
                            lhsT=xT[u * 64 : (u + 1) * 64, gl, :],
                            rhs=hp[u * 64 : (u + 1) * 64, g, :],
                            start=True,
                            stop=True,
                        )
                    o_sb = outp.tile([128, D], FP32, tag="o_sb")
                    nc.scalar.activation(
                        out=o_sb[:, :],
                        in_=o_ps[:, :],
                        func=AF.Copy,
                        scale=rinv[:, gl : gl + 1],
                    )
                    nc.sync.dma_start(
                        out=out_d.ap().rearrange("b i d -> (b i) d")[
                            128 * g : 128 * (g + 1), :
                        ],
                        in_=o_sb[:, :],
                    )
    nc.compile()
    return nc


_CACHE = {}


def _compiled():
    if "nc" not in _CACHE:
        _CACHE["nc"] = build_nc()
    return _CACHE["nc"]


def _shard_inputs(inputs, adj, embedding, attn_a):
    inputs = np.asarray(inputs)
    adj = np.asarray(adj)
    emb16 = np.ascontiguousarray(np.asarray(embedding, dtype=np.float32).astype(BF))
    attnT = np.ascontiguousarray(np.asarray(attn_a, dtype=np.float32).T)  # [D, 4]
    in_maps = []
    for c in range(NCORES):
        sl = slice(c * BC, (c + 1) * BC)
        # idx[(u,i), g] = inputs[c*BC + 2g+u, i]
        idx = np.ascontiguousarray(
            inputs[sl].reshape(NPAIR, 2, N).transpose(1, 2, 0).reshape(128, NPAIR)
            .astype(np.int32)
        )
        adj_r = adj[sl].reshape(NPAIR, 2, N, N).astype(np.int32)  # [g, u, i, j]
        A = adj_r.transpose(1, 2, 0, 3).reshape(128, NPAIR, N)  # [(u,i), g, j]
        Bm = adj_r.transpose(1, 3, 0, 2).reshape(128, NPAIR, N)  # [(v,j), g, i]
        mx = (
            np.stack([A == k + 1 for k in range(4)], axis=1)
            .astype(np.uint8)
            .reshape(128, 4 * NPAIR * N)
        )
        mxT = (
            np.stack([Bm == k + 1 for k in range(4)], axis=1)
            .astype(np.uint8)
            .reshape(128, 4 * NPAIR * N)
        )
        in_maps.append(
            dict(emb=emb16, idx=idx, mx=np.ascontiguousarray(mx),
                 mxT=np.ascontiguousarray(mxT), attnT=attnT)
        )
    return in_maps


def kernel(inputs, adj, mask_item, item, embedding, attn_a):
    in_maps = _shard_inputs(inputs, adj, embedding, attn_a)
    res = run_bass_kernel_spmd(
        _compiled(), in_maps, core_ids=list(range(NCORES))
    ).results
    out = np.concatenate([np.asarray(res[c]["out"]) for c in range(NCORES)], axis=0)
    return out.astype(np.float32)


# revision 9
# speedup vs baseline: 1.8030x; 1.0351x over previous
"""Trainium2 Bass kernel for GCE-GNN LocalAggregator (gnn_message_passing).

Computes, for each batch b:
    h = embedding[inputs]                            # [N, D] gather
    e_k = leakyrelu((h * a_k) @ h.T, 0.2)            # k = 0..3
    alpha = softmax(where(adj == k+1, e_k, -inf))    # edge-type select
    out = alpha @ h

Sharding: data-parallel over batch B=512 across 8 cores (64 batches/core).
The embedding table is staged in bf16 (11MB) per core; only looked-up rows
are read from HBM.

Key structural ideas vs the naive mapping:
  * One indirect DMA per PAIR gathers 128 rows (one per partition) into the
    pair layout h_pair[(u,i), g, d] - 32 gathers instead of 64, bf16.
  * hT (d on partitions) comes from 32 PE transposes of whole pairs.
  * e_k is SYMMETRIC in (i,j) (e_k[i,j] = sum_d h_i h_j a_k). The transposed
    attention matrix xT (the out-matmul lhsT) is read from the SAME e buffer
    with host-transposed one-hot masks - no transpose of x at all.
  * Edge-type selection is multiply-by-one-hot + reduce over k (k is the
    INNERMOST e dimension, arranged via the matmul rhs AP dim order), so
    there is no NEG-background memset and no copy_predicated. The -1e9
    no-edge background rides the exp's elementwise bias operand on the
    Scalar engine: exp(scale*e_sel + B0) is exact 0 for no-edge entries.
  * exp(leakyrelu(x)) == max(exp(x), exp(0.2x)).
  * bf16 everywhere off-PSUM; chunks of 8 pairs pipeline across engines,
    with program order interleaved so the in-order PE queue never parks
    later-chunk transposes in front of ready e-matmuls.
"""

import os
import sys

import numpy as np

for _p in ("/opt/trn_rl_repo",):
    if _p not in sys.path and os.path.isdir(_p):
        sys.path.insert(0, _p)

import ml_dtypes

import concourse.bass as bass
import concourse.bacc as bacc
import concourse.tile as tile
from concourse import mybir
from concourse.bass_utils import run_bass_kernel_spmd
from concourse.masks import make_identity

B, N, D, V = 512, 64, 128, 43098
NCORES = 8
BC = B // NCORES          # 64 local batches per core
NPAIR = BC // 2           # 32 pairs
ALPHA = 0.2
NEG_BIG = -1.0e9          # exp(NEG_BIG) == 0; stands in for -9e15
CH = 8                    # pairs per pipeline chunk
NCH = NPAIR // CH

FP32 = mybir.dt.float32
BF16 = mybir.dt.bfloat16
I32 = mybir.dt.int32
AF = mybir.ActivationFunctionType
OP = mybir.AluOpType
AX = mybir.AxisListType

BF = ml_dtypes.bfloat16


def build_nc():
    nc = bacc.Bacc("TRN2", target_bir_lowering=False, debug=False)

    emb_d = nc.dram_tensor("emb", [V, D], BF16, kind="ExternalInput")
    idx_d = nc.dram_tensor("idx", [128, NPAIR], I32, kind="ExternalInput")
    m2x_d = nc.dram_tensor("m2x", [128, NPAIR * N * 4], BF16, kind="ExternalInput")
    m2t_d = nc.dram_tensor("m2t", [128, NPAIR * N * 4], BF16, kind="ExternalInput")
    emx_d = nc.dram_tensor("emx", [128, NPAIR * N], BF16, kind="ExternalInput")
    emt_d = nc.dram_tensor("emt", [128, NPAIR * N], BF16, kind="ExternalInput")
    attn_d = nc.dram_tensor("attnT", [D, 4], FP32, kind="ExternalInput")
    out_d = nc.dram_tensor("out", [BC, N, D], FP32, kind="ExternalOutput")

    with tile.TileContext(nc) as tc:
        with (
            tc.tile_pool(name="singles", bufs=1) as singles,
            tc.tile_pool(name="big", bufs=1) as big,
            tc.tile_pool(name="chnk", bufs=3) as chnk,
            tc.tile_pool(name="outp", bufs=3) as outp,
            tc.tile_pool(name="ps_t", bufs=2, space="PSUM") as ps_t,
            tc.tile_pool(name="ps_e", bufs=3, space="PSUM") as ps_e,
            tc.tile_pool(name="ps_o", bufs=3, space="PSUM") as ps_o,
        ):
            # ---- tiny inputs ----
            idx_sb = singles.tile([128, NPAIR], I32)
            nc.sync.dma_start(out=idx_sb[:, :], in_=idx_d[:, :])
            attn_sb = singles.tile([128, 4], FP32)
            nc.sync.dma_start(out=attn_sb[:, :], in_=attn_d[:, :])
            ident = singles.tile([128, 128], BF16)
            make_identity(nc, ident[:, :])

            # one-hot selection masks [p, g, j, k] bf16 and exp biases [p, g, j]
            m2x_sb = big.tile([128, NPAIR, N, 4], BF16, tag="m2x")
            nc.sync.dma_start(
                out=m2x_sb[:, :, :, :],
                in_=m2x_d.ap().rearrange("p (g j k) -> p g j k", g=NPAIR, j=N),
            )
            m2t_sb = big.tile([128, NPAIR, N, 4], BF16, tag="m2t")
            nc.sync.dma_start(
                out=m2t_sb[:, :, :, :],
                in_=m2t_d.ap().rearrange("p (g j k) -> p g j k", g=NPAIR, j=N),
            )
            emx_sb = big.tile([128, NPAIR, N], BF16, tag="emx")
            nc.sync.dma_start(
                out=emx_sb[:, :, :],
                in_=emx_d.ap().rearrange("p (g j) -> p g j", g=NPAIR),
            )
            emt_sb = big.tile([128, NPAIR, N], BF16, tag="emt")
            nc.sync.dma_start(
                out=emt_sb[:, :, :],
                in_=emt_d.ap().rearrange("p (g j) -> p g j", g=NPAIR),
            )

            # h_pair[(u,i), g, d], hT[d, g, (u,i)], S[d, k, g, (u,i)]
            hp = big.tile([128, NPAIR, D], BF16, tag="hp")
            hT = big.tile([128, NPAIR, 128], BF16, tag="hT")
            S_all = big.tile([128, 4, NPAIR, 128], BF16, tag="S")

            flat = "p g j -> p (g j)"
            chunk_state = {}

            def emit_ingest(c):
                """Gather + transpose + S for pair-group c."""
                gs = slice(c * CH, (c + 1) * CH)
                for g in range(c * CH, (c + 1) * CH):
                    nc.gpsimd.indirect_dma_start(
                        out=hp[:, g, :],
                        out_offset=None,
                        in_=emb_d[:, :],
                        in_offset=bass.IndirectOffsetOnAxis(
                            ap=idx_sb[:, g : g + 1], axis=0
                        ),
                    )
                for g in range(c * CH, (c + 1) * CH):
                    t_ps = ps_t.tile([128, 128], BF16, tag="t_ps")
                    nc.tensor.transpose(
                        out=t_ps[:, :], in_=hp[:, g, :], identity=ident[:, :]
                    )
                    if g % 2 == 0:
                        nc.scalar.copy(out=hT[:, g, :], in_=t_ps[:, :])
                    else:
                        nc.vector.tensor_copy(out=hT[:, g, :], in_=t_ps[:, :])
                for k in range(4):
                    nc.vector.tensor_scalar_mul(
                        out=S_all[:, k, gs, :].rearrange("p g q -> p (g q)"),
                        in0=hT[:, gs, :].rearrange("p g q -> p (g q)"),
                        scalar1=attn_sb[:, k : k + 1],
                    )

            def emit_emm_select(c):
                """e matmuls + one-hot select + exp + row sums for chunk c."""
                g0 = c * CH
                gs = slice(g0, g0 + CH)
                e_c = chnk.tile([128, CH, N, 4], BF16, tag="e")
                me = chnk.tile([128, CH, N, 4], BF16, tag="me")
                meT = chnk.tile([128, CH, N, 4], BF16, tag="meT")
                red = chnk.tile([128, CH, N], FP32, tag="red")
                redT = chnk.tile([128, CH, N], FP32, tag="redT")
                xe1 = chnk.tile([128, CH, N], BF16, tag="xe1")
                xe2 = chnk.tile([128, CH, N], BF16, tag="xe2")
                xe3 = chnk.tile([128, CH, N], BF16, tag="xe3")
                xe4 = chnk.tile([128, CH, N], BF16, tag="xe4")
                xs = chnk.tile([128, CH, N], BF16, tag="xs")
                xT = chnk.tile([128, CH, N], BF16, tag="xT")
                ssum = chnk.tile([128, CH], FP32, tag="ssum")
                rinv = chnk.tile([128, CH], FP32, tag="rinv")
                chunk_state[c] = (xT, rinv)

                # e matmuls; rhs dim order (j, k) puts k innermost
                for gl in range(CH):
                    g = g0 + gl
                    e_ps = ps_e.tile([128, N, 4], FP32, tag="e_ps")
                    for u in range(2):
                        nc.tensor.matmul(
                            out=e_ps[u * 64 : (u + 1) * 64, :, :],
                            lhsT=hT[:, g, u * 64 : (u + 1) * 64],
                            rhs=S_all[:, :, g, u * 64 : (u + 1) * 64]
                            .rearrange("p k j -> p j k"),
                            start=True,
                            stop=True,
                        )
                    if gl % 2 == 0:
                        nc.scalar.copy(out=e_c[:, gl, :, :], in_=e_ps[:, :, :])
                    else:
                        nc.vector.tensor_copy(out=e_c[:, gl, :, :], in_=e_ps[:, :, :])

                # select: multiply one-hot, reduce over k (symmetric e trick)
                nc.vector.tensor_tensor(
                    out=me[:, :, :, :], in0=e_c[:, :, :, :],
                    in1=m2x_sb[:, gs, :, :], op=OP.mult,
                )
                nc.vector.reduce_sum(out=red[:, :, :], in_=me[:, :, :, :], axis=AX.X)
                nc.vector.tensor_tensor(
                    out=meT[:, :, :, :], in0=e_c[:, :, :, :],
                    in1=m2t_sb[:, gs, :, :], op=OP.mult,
                )
                nc.vector.reduce_sum(out=redT[:, :, :], in_=meT[:, :, :, :], axis=AX.X)

                # x = exp(leakyrelu(.)) = max(exp(.), exp(0.2 .)); no-edge
                # entries are zeroed exactly by the post-exp edge-mask multiply
                nc.scalar.activation(
                    out=xe1[:, :, :].rearrange(flat),
                    in_=red[:, :, :].rearrange(flat),
                    func=AF.Exp,
                )
                nc.scalar.activation(
                    out=xe2[:, :, :].rearrange(flat),
                    in_=red[:, :, :].rearrange(flat),
                    func=AF.Exp,
                    scale=ALPHA,
                )
                nc.vector.tensor_tensor(
                    out=xe1[:, :, :], in0=xe1[:, :, :], in1=xe2[:, :, :], op=OP.max
                )
                nc.vector.tensor_tensor(
                    out=xs[:, :, :], in0=xe1[:, :, :],
                    in1=emx_sb[:, gs, :], op=OP.mult,
                )
                nc.scalar.activation(
                    out=xe3[:, :, :].rearrange(flat),
                    in_=redT[:, :, :].rearrange(flat),
                    func=AF.Exp,
                )
                nc.scalar.activation(
                    out=xe4[:, :, :].rearrange(flat),
                    in_=redT[:, :, :].rearrange(flat),
                    func=AF.Exp,
                    scale=ALPHA,
                )
                nc.vector.tensor_tensor(
                    out=xe3[:, :, :], in0=xe3[:, :, :], in1=xe4[:, :, :], op=OP.max
                )
                nc.vector.tensor_tensor(
                    out=xT[:, :, :], in0=xe3[:, :, :],
                    in1=emt_sb[:, gs, :], op=OP.mult,
                )

                nc.vector.reduce_sum(out=ssum[:, :], in_=xs[:, :, :], axis=AX.X)
                nc.vector.reciprocal(out=rinv[:, :], in_=ssum[:, :])

            def emit_out(c):
                """out matmuls + scaled evac + DMA for chunk c."""
                xT, rinv = chunk_state.pop(c)
                g0 = c * CH
                for gl in range(CH):
                    g = g0 + gl
                    o_ps = ps_o.tile([128, D], FP32, tag="o_ps")
                    for u in range(2):
                        nc.tensor.matmul(
                            out=o_ps[u * 64 : (u + 1) * 64, :],
                            lhsT=xT[u * 64 : (u + 1) * 64, gl, :],
                            rhs=hp[u * 64 : (u + 1) * 64, g, :],
                            start=True,
                            stop=True,
                        )
                    o_sb = outp.tile([128, D], FP32, tag="o_sb")
                    nc.scalar.activation(
                        out=o_sb[:, :],
                        in_=o_ps[:, :],
                        func=AF.Copy,
                        scale=rinv[:, gl : gl + 1],
                    )
                    nc.sync.dma_start(
                        out=out_d.ap().rearrange("b i d -> (b i) d")[
                            128 * g : 128 * (g + 1), :
                        ],
                        in_=o_sb[:, :],
                    )

            # Software pipeline: ingest(c) -> e/select(c), with out(c-1)
            # interleaved so the PE has ready work while chunk c's exps run.
            emit_ingest(0)
            for c in range(NCH):
                if c + 1 < NCH:
                    emit_ingest(c + 1)
                emit_emm_select(c)
                if c >= 1:
                    emit_out(c - 1)
            emit_out(NCH - 1)
    nc.compile()
    return nc


_CACHE = {}


def _compiled():
    if "nc" not in _CACHE:
        _CACHE["nc"] = build_nc()
    return _CACHE["nc"]


def _shard_inputs(inputs, adj, embedding, attn_a):
    inputs = np.asarray(inputs)
    adj = np.asarray(adj)
    emb16 = np.ascontiguousarray(np.asarray(embedding, dtype=np.float32).astype(BF))
    attnT = np.ascontiguousarray(np.asarray(attn_a, dtype=np.float32).T)  # [D, 4]
    in_maps = []
    for c in range(NCORES):
        sl = slice(c * BC, (c + 1) * BC)
        # idx[(u,i), g] = inputs[c*BC + 2g+u, i]
        idx = np.ascontiguousarray(
            inputs[sl].reshape(NPAIR, 2, N).transpose(1, 2, 0).reshape(128, NPAIR)
            .astype(np.int32)
        )
        adj_r = adj[sl].reshape(NPAIR, 2, N, N).astype(np.int32)  # [g, u, i, j]
        A = adj_r.transpose(1, 2, 0, 3).reshape(128, NPAIR, N)  # [(u,i), g, j]
        Bm = adj_r.transpose(1, 3, 0, 2).reshape(128, NPAIR, N)  # [(v,j), g, i]
        # one-hot over k (innermost), bf16
        m2x = np.ascontiguousarray(
            np.stack([A == k + 1 for k in range(4)], axis=-1)
            .astype(BF).reshape(128, NPAIR * N * 4)
        )
        m2t = np.ascontiguousarray(
            np.stack([Bm == k + 1 for k in range(4)], axis=-1)
            .astype(BF).reshape(128, NPAIR * N * 4)
        )
        # edge masks: 1 where there is an edge, 0 where not
        emx = np.ascontiguousarray((A > 0).astype(BF).reshape(128, NPAIR * N))
        emt = np.ascontiguousarray((Bm > 0).astype(BF).reshape(128, NPAIR * N))
        in_maps.append(
            dict(emb=emb16, idx=idx, m2x=m2x, m2t=m2t, emx=emx, emt=emt, attnT=attnT)
        )
    return in_maps


def kernel(inputs, adj, mask_item, item, embedding, attn_a):
    in_maps = _shard_inputs(inputs, adj, embedding, attn_a)
    res = run_bass_kernel_spmd(
        _compiled(), in_maps, core_ids=list(range(NCORES))
    ).results
    out = np.concatenate([np.asarray(res[c]["out"]) for c in range(NCORES)], axis=0)
    return out.astype(np.float32)


# revision 11
# speedup vs baseline: 2.0547x; 1.1396x over previous
"""Trainium2 Bass kernel for GCE-GNN LocalAggregator (gnn_message_passing).

Computes, for each batch b:
    h = embedding[inputs]                            # [N, D] gather
    e_k = leakyrelu((h * a_k) @ h.T, 0.2)            # k = 0..3
    alpha = softmax(where(adj == k+1, e_k, -inf))    # edge-type select
    out = alpha @ h

Sharding: data-parallel over batch B=512 across 8 cores (64 batches/core).
The embedding table is staged in bf16 (11MB) per core; only looked-up rows
are read from HBM.

Key structural ideas vs the naive mapping:
  * One indirect DMA per PAIR gathers 128 rows (one per partition) into the
    pair layout h_pair[(u,i), g, d] - 32 gathers instead of 64, bf16.
  * hT (d on partitions) comes from 32 PE transposes of whole pairs.
  * e_k is SYMMETRIC in (i,j) (e_k[i,j] = sum_d h_i h_j a_k), so ONLY the
    transposed attention matrix xT is ever materialized, selected straight
    from the e buffer with host-transposed one-hot masks ([(v,j), g, i]
    reading of the same bytes). No transpose of x, no untransposed x at all.
  * h_pair is padded with a ones column (the per-pair gather leaves the
    129-col layout contiguous per instruction), so the out-matmul's last
    column yields the softmax row sums for free - the whole untransposed
    selection/exp pipeline and its masks are gone.
  * Edge-type selection is multiply-by-one-hot + reduce over k (k is the
    INNERMOST e dimension via the matmul rhs AP dim order); no-edge entries
    are zeroed exactly by a post-exp edge-mask multiply.
  * exp(leakyrelu(x)) == max(exp(x), exp(0.2x)).
  * bf16 everywhere off-PSUM; chunks of 8 pairs pipeline across engines,
    with program order interleaved so the in-order PE queue never parks
    later-chunk transposes in front of ready e-matmuls.
"""

import os
import sys

import numpy as np

for _p in ("/opt/trn_rl_repo",):
    if _p not in sys.path and os.path.isdir(_p):
        sys.path.insert(0, _p)

import ml_dtypes

import concourse.bass as bass
import concourse.bacc as bacc
import concourse.tile as tile
from concourse import mybir
from concourse.bass_utils import run_bass_kernel_spmd

B, N, D, V = 512, 64, 128, 43098
NCORES = 8
BC = B // NCORES          # 64 local batches per core
NPAIR = BC // 2           # 32 pairs
ALPHA = 0.2
CH = 8                    # pairs per pipeline chunk
NCH = NPAIR // CH

FP32 = mybir.dt.float32
BF16 = mybir.dt.bfloat16
I32 = mybir.dt.int32
AF = mybir.ActivationFunctionType
OP = mybir.AluOpType
AX = mybir.AxisListType

BF = ml_dtypes.bfloat16


def build_nc():
    nc = bacc.Bacc("TRN2", target_bir_lowering=False, debug=False)

    emb_d = nc.dram_tensor("emb", [V, D], BF16, kind="ExternalInput")
    idx_d = nc.dram_tensor("idx", [128, NPAIR], I32, kind="ExternalInput")
    ident_d = nc.dram_tensor("ident", [128, 128], BF16, kind="ExternalInput")
    m2t_d = nc.dram_tensor("m2t", [128, NPAIR * N * 4], BF16, kind="ExternalInput")
    emt_d = nc.dram_tensor("emt", [128, NPAIR * N], BF16, kind="ExternalInput")
    attn_d = nc.dram_tensor("attnT", [D, 4], FP32, kind="ExternalInput")
    out_d = nc.dram_tensor("out", [BC, N, D], FP32, kind="ExternalOutput")

    with tile.TileContext(nc) as tc:
        with (
            tc.tile_pool(name="singles", bufs=1) as singles,
            tc.tile_pool(name="big", bufs=1) as big,
            tc.tile_pool(name="chnk", bufs=3) as chnk,
            tc.tile_pool(name="outp", bufs=4) as outp,
            tc.tile_pool(name="ps_t", bufs=2, space="PSUM") as ps_t,
            tc.tile_pool(name="ps_e", bufs=3, space="PSUM") as ps_e,
            tc.tile_pool(name="ps_o", bufs=3, space="PSUM") as ps_o,
        ):
            # ---- tiny inputs (idx first: the gathers gate on it) ----
            idx_sb = singles.tile([128, NPAIR], I32)
            nc.sync.dma_start(out=idx_sb[:, :], in_=idx_d[:, :])
            attn_sb = singles.tile([128, 4], FP32)
            nc.sync.dma_start(out=attn_sb[:, :], in_=attn_d[:, :])
            ident = singles.tile([128, 128], BF16)
            nc.sync.dma_start(out=ident[:, :], in_=ident_d[:, :])

            # transposed one-hot masks [p, g, i, k] bf16 + edge mask [p, g, i]
            m2t_sb = big.tile([128, NPAIR, N, 4], BF16, tag="m2t")
            nc.sync.dma_start(
                out=m2t_sb[:, :, :, :],
                in_=m2t_d.ap().rearrange("p (g j k) -> p g j k", g=NPAIR, j=N),
            )
            emt_sb = big.tile([128, NPAIR, N], BF16, tag="emt")
            nc.sync.dma_start(
                out=emt_sb[:, :, :],
                in_=emt_d.ap().rearrange("p (g j) -> p g j", g=NPAIR),
            )

            # h_pair[(u,i), g, d | 1], hT[d, g, (u,i)], S[d, k, g, (u,i)]
            hp = big.tile([128, NPAIR, D + 1], BF16, tag="hp")
            hT = big.tile([128, NPAIR, 128], BF16, tag="hT")
            S_all = big.tile([128, 4, NPAIR, 128], BF16, tag="S")

            # ones column for the row-sum trick
            nc.vector.memset(hp[:, :, D : D + 1], 1.0)

            flat = "p g j -> p (g j)"
            chunk_state = {}

            def emit_ingest(c):
                """Gather + transpose + S for pair-group c."""
                gs = slice(c * CH, (c + 1) * CH)
                for g in range(c * CH, (c + 1) * CH):
                    nc.gpsimd.indirect_dma_start(
                        out=hp[:, g, 0:D],
                        out_offset=None,
                        in_=emb_d[:, :],
                        in_offset=bass.IndirectOffsetOnAxis(
                            ap=idx_sb[:, g : g + 1], axis=0
                        ),
                    )
                for g in range(c * CH, (c + 1) * CH):
                    t_ps = ps_t.tile([128, 128], BF16, tag="t_ps")
                    nc.tensor.transpose(
                        out=t_ps[:, :], in_=hp[:, g, 0:D], identity=ident[:, :]
                    )
                    if g % 2 == 0:
                        nc.scalar.copy(out=hT[:, g, :], in_=t_ps[:, :])
                    else:
                        nc.vector.tensor_copy(out=hT[:, g, :], in_=t_ps[:, :])
                for k in range(4):
                    if k % 2 == 0:
                        nc.vector.tensor_scalar_mul(
                            out=S_all[:, k, gs, :].rearrange("p g q -> p (g q)"),
                            in0=hT[:, gs, :].rearrange("p g q -> p (g q)"),
                            scalar1=attn_sb[:, k : k + 1],
                        )
                    else:
                        nc.scalar.activation(
                            out=S_all[:, k, gs, :].rearrange("p g q -> p (g q)"),
                            in_=hT[:, gs, :].rearrange("p g q -> p (g q)"),
                            func=AF.Copy,
                            scale=attn_sb[:, k : k + 1],
                        )

            def emit_emm_select(c):
                """e matmuls + one-hot select + exp for chunk c (xT only)."""
                g0 = c * CH
                gs = slice(g0, g0 + CH)
                e_c = chnk.tile([128, CH, N, 4], BF16, tag="e")
                meT = chnk.tile([128, CH, N, 4], BF16, tag="meT")
                redT = chnk.tile([128, CH, N], BF16, tag="redT")
                xe3 = chnk.tile([128, CH, N], BF16, tag="xe3")
                xe4 = chnk.tile([128, CH, N], BF16, tag="xe4")
                xT = chnk.tile([128, CH, N], BF16, tag="xT")
                chunk_state[c] = xT

                # e matmuls; rhs dim order (j, k) puts k innermost
                for gl in range(CH):
                    g = g0 + gl
                    e_ps = ps_e.tile([128, N, 4], FP32, tag="e_ps")
                    for u in range(2):
                        nc.tensor.matmul(
                            out=e_ps[u * 64 : (u + 1) * 64, :, :],
                            lhsT=hT[:, g, u * 64 : (u + 1) * 64],
                            rhs=S_all[:, :, g, u * 64 : (u + 1) * 64]
                            .rearrange("p k j -> p j k"),
                            start=True,
                            stop=True,
                        )
                    if gl % 2 == 0:
                        nc.scalar.copy(out=e_c[:, gl, :, :], in_=e_ps[:, :, :])
                    else:
                        nc.vector.tensor_copy(out=e_c[:, gl, :, :], in_=e_ps[:, :, :])

                # transposed select: multiply one-hot, reduce over k
                # (e_k symmetric => same e bytes serve the [(v,j), g, i] view)
                nc.vector.tensor_tensor(
                    out=meT[:, :, :, :], in0=e_c[:, :, :, :],
                    in1=m2t_sb[:, gs, :, :], op=OP.mult,
                )
                # exact: per (p,g,i) the k-sum has one nonzero term (one-hot)
                with nc.allow_low_precision(reason="one-hot k-select, sum is exact"):
                    nc.vector.reduce_sum(
                        out=redT[:, :, :], in_=meT[:, :, :, :], axis=AX.X
                    )

                # xT = exp(leakyrelu(.)) = max(exp(.), exp(0.2 .)); no-edge
                # entries zeroed exactly by the post-exp edge-mask multiply
                nc.scalar.activation(
                    out=xe3[:, :, :].rearrange(flat),
                    in_=redT[:, :, :].rearrange(flat),
                    func=AF.Exp,
                )
                nc.scalar.activation(
                    out=xe4[:, :, :].rearrange(flat),
                    in_=redT[:, :, :].rearrange(flat),
                    func=AF.Exp,
                    scale=ALPHA,
                )
                nc.vector.tensor_tensor(
                    out=xe3[:, :, :], in0=xe3[:, :, :], in1=xe4[:, :, :], op=OP.max
                )
                nc.vector.tensor_tensor(
                    out=xT[:, :, :], in0=xe3[:, :, :],
                    in1=emt_sb[:, gs, :], op=OP.mult,
                )

            def emit_out(c):
                """out matmuls (ones column -> row sums) + scaled evac + DMA."""
                xT = chunk_state.pop(c)
                g0 = c * CH
                for gl in range(CH):
                    g = g0 + gl
                    o_ps = ps_o.tile([128, D + 1], FP32, tag="o_ps")
                    for u in range(2):
                        nc.tensor.matmul(
                            out=o_ps[u * 64 : (u + 1) * 64, :],
                            lhsT=xT[u * 64 : (u + 1) * 64, gl, :],
                            rhs=hp[u * 64 : (u + 1) * 64, g, :],
                            start=True,
                            stop=True,
                        )
                    rinv1 = outp.tile([128, 1], FP32, tag="rinv1")
                    nc.vector.reciprocal(out=rinv1[:, :], in_=o_ps[:, D : D + 1])
                    o_sb = outp.tile([128, D], FP32, tag="o_sb")
                    nc.scalar.activation(
                        out=o_sb[:, :],
                        in_=o_ps[:, 0:D],
                        func=AF.Copy,
                        scale=rinv1[:, :],
                    )
                    nc.sync.dma_start(
                        out=out_d.ap().rearrange("b i d -> (b i) d")[
                            128 * g : 128 * (g + 1), :
                        ],
                        in_=o_sb[:, :],
                    )

            # Software pipeline: ingest(c) -> e/select(c), with out(c-1)
            # interleaved so the PE has ready work while chunk c's exps run.
            emit_ingest(0)
            for c in range(NCH):
                if c + 1 < NCH:
                    emit_ingest(c + 1)
                emit_emm_select(c)
                if c >= 1:
                    emit_out(c - 1)
            emit_out(NCH - 1)
    nc.compile()
    return nc


_CACHE = {}


def _compiled():
    if "nc" not in _CACHE:
        _CACHE["nc"] = build_nc()
    return _CACHE["nc"]


def _shard_inputs(inputs, adj, embedding, attn_a):
    inputs = np.asarray(inputs)
    adj = np.asarray(adj)
    emb16 = np.ascontiguousarray(np.asarray(embedding, dtype=np.float32).astype(BF))
    attnT = np.ascontiguousarray(np.asarray(attn_a, dtype=np.float32).T)  # [D, 4]
    ident = np.ascontiguousarray(np.eye(128).astype(BF))
    in_maps = []
    for c in range(NCORES):
        sl = slice(c * BC, (c + 1) * BC)
        # idx[(u,i), g] = inputs[c*BC + 2g+u, i]
        idx = np.ascontiguousarray(
            inputs[sl].reshape(NPAIR, 2, N).transpose(1, 2, 0).reshape(128, NPAIR)
            .astype(np.int32)
        )
        adj_r = adj[sl].reshape(NPAIR, 2, N, N).astype(np.int32)  # [g, u, i, j]
        Bm = adj_r.transpose(1, 3, 0, 2).reshape(128, NPAIR, N)  # [(v,j), g, i]
        # transposed one-hot over k (innermost), bf16
        m2t = np.ascontiguousarray(
            np.stack([Bm == k + 1 for k in range(4)], axis=-1)
            .astype(BF).reshape(128, NPAIR * N * 4)
        )
        # edge mask: 1 where there is an edge, 0 where not
        emt = np.ascontiguousarray((Bm > 0).astype(BF).reshape(128, NPAIR * N))
        in_maps.append(
            dict(emb=emb16, idx=idx, ident=ident, m2t=m2t, emt=emt, attnT=attnT)
        )
    return in_maps


def kernel(inputs, adj, mask_item, item, embedding, attn_a):
    in_maps = _shard_inputs(inputs, adj, embedding, attn_a)
    res = run_bass_kernel_spmd(
        _compiled(), in_maps, core_ids=list(range(NCORES))
    ).results
    out = np.concatenate([np.asarray(res[c]["out"]) for c in range(NCORES)], axis=0)
    return out.astype(np.float32)


# revision 12
# speedup vs baseline: 2.1339x; 1.0385x over previous
"""Trainium2 Bass kernel for GCE-GNN LocalAggregator (gnn_message_passing).

Computes, for each batch b:
    h = embedding[inputs]                            # [N, D] gather
    e_k = leakyrelu((h * a_k) @ h.T, 0.2)            # k = 0..3
    alpha = softmax(where(adj == k+1, e_k, -inf))    # edge-type select
    out = alpha @ h

Sharding: data-parallel over batch B=512 across 8 cores (64 batches/core).
The embedding table is staged in bf16 (11MB) per core; only looked-up rows
are read from HBM.

Key structural ideas vs the naive mapping:
  * One indirect DMA per PAIR gathers 128 rows (one per partition) into the
    pair layout h_pair[(u,i), g, d] - 32 gathers instead of 64, bf16.
  * hT (d on partitions) comes from 32 PE transposes of whole pairs.
  * e_k is SYMMETRIC in (i,j) (e_k[i,j] = sum_d h_i h_j a_k), so ONLY the
    transposed attention matrix xT is ever materialized, selected straight
    from the e buffer with host-transposed one-hot masks ([(v,j), g, i]
    reading of the same bytes). No transpose of x, no untransposed x at all.
  * h_pair is padded with a ones column (the per-pair gather leaves the
    129-col layout contiguous per instruction), so the out-matmul's last
    column yields the softmax row sums for free - the whole untransposed
    selection/exp pipeline and its masks are gone.
  * Edge-type selection is multiply-by-one-hot + reduce over k (k is the
    INNERMOST e dimension via the matmul rhs AP dim order); no-edge entries
    are zeroed exactly by a post-exp edge-mask multiply.
  * exp(leakyrelu(x)) == max(exp(x), exp(0.2x)).
  * bf16 everywhere off-PSUM; chunks of 8 pairs pipeline across engines,
    with program order interleaved so the in-order PE queue never parks
    later-chunk transposes in front of ready e-matmuls.
"""

import os
import sys

import numpy as np

for _p in ("/opt/trn_rl_repo",):
    if _p not in sys.path and os.path.isdir(_p):
        sys.path.insert(0, _p)

import ml_dtypes

import concourse.bass as bass
import concourse.bacc as bacc
import concourse.tile as tile
from concourse import mybir
from concourse.bass_utils import run_bass_kernel_spmd

B, N, D, V = 512, 64, 128, 43098
NCORES = 8
BC = B // NCORES          # 64 local batches per core
NPAIR = BC // 2           # 32 pairs
ALPHA = 0.2
NEG_BIG = -1.0e9          # exp(NEG_BIG) == 0; stands in for -9e15
CHUNKS = [(0, 8), (8, 8), (16, 8), (24, 4), (28, 4)]  # (start pair, n pairs)
NCH = len(CHUNKS)

FP32 = mybir.dt.float32
BF16 = mybir.dt.bfloat16
I32 = mybir.dt.int32
AF = mybir.ActivationFunctionType
OP = mybir.AluOpType
AX = mybir.AxisListType

BF = ml_dtypes.bfloat16


def build_nc():
    nc = bacc.Bacc("TRN2", target_bir_lowering=False, debug=False)

    emb_d = nc.dram_tensor("emb", [V, D], BF16, kind="ExternalInput")
    idx_d = nc.dram_tensor("idx", [128, NPAIR], I32, kind="ExternalInput")
    ident_d = nc.dram_tensor("ident", [128, 128], BF16, kind="ExternalInput")
    mt_d = nc.dram_tensor("mt", [128, 4 * NPAIR * N], mybir.dt.uint8, kind="ExternalInput")
    attn_d = nc.dram_tensor("attnT", [D, 4], FP32, kind="ExternalInput")
    out_d = nc.dram_tensor("out", [BC, N, D], FP32, kind="ExternalOutput")

    with tile.TileContext(nc) as tc:
        with (
            tc.tile_pool(name="singles", bufs=1) as singles,
            tc.tile_pool(name="big", bufs=1) as big,
            tc.tile_pool(name="chnk", bufs=3) as chnk,
            tc.tile_pool(name="outp", bufs=4) as outp,
            tc.tile_pool(name="ps_t", bufs=2, space="PSUM") as ps_t,
            tc.tile_pool(name="ps_e", bufs=3, space="PSUM") as ps_e,
            tc.tile_pool(name="ps_o", bufs=3, space="PSUM") as ps_o,
        ):
            # ---- tiny inputs (idx first: the gathers gate on it) ----
            idx_sb = singles.tile([128, NPAIR], I32)
            nc.sync.dma_start(out=idx_sb[:, :], in_=idx_d[:, :])
            attn_sb = singles.tile([128, 4], FP32)
            nc.sync.dma_start(out=attn_sb[:, :], in_=attn_d[:, :])
            ident = singles.tile([128, 128], BF16)
            nc.sync.dma_start(out=ident[:, :], in_=ident_d[:, :])

            # transposed edge-type masks [p, k, g, i] uint8
            mt_sb = big.tile([128, 4, NPAIR, N], mybir.dt.uint8, tag="mt")
            nc.sync.dma_start(
                out=mt_sb[:, :, :, :],
                in_=mt_d.ap().rearrange("p (k g j) -> p k g j", k=4, j=N),
            )

            # h_pair[(u,i), g, d | 1], hT[d, g, (u,i)], S[d, k, g, (u,i)]
            hp = big.tile([128, NPAIR, D + 1], BF16, tag="hp")
            hT = big.tile([128, NPAIR, 128], BF16, tag="hT")
            S_all = big.tile([128, 4, NPAIR, 128], BF16, tag="S")

            # ones column for the row-sum trick
            nc.vector.memset(hp[:, :, D : D + 1], 1.0)

            flat = "p g j -> p (g j)"
            chunk_state = {}

            def emit_ingest(c):
                """Gather + transpose + S for pair-group c."""
                g0, ch = CHUNKS[c]
                gs = slice(g0, g0 + ch)
                for g in range(g0, g0 + ch):
                    nc.gpsimd.indirect_dma_start(
                        out=hp[:, g, 0:D],
                        out_offset=None,
                        in_=emb_d[:, :],
                        in_offset=bass.IndirectOffsetOnAxis(
                            ap=idx_sb[:, g : g + 1], axis=0
                        ),
                    )
                for g in range(g0, g0 + ch):
                    t_ps = ps_t.tile([128, 128], BF16, tag="t_ps")
                    nc.tensor.transpose(
                        out=t_ps[:, :], in_=hp[:, g, 0:D], identity=ident[:, :]
                    )
                    if g % 2 == 0:
                        nc.scalar.copy(out=hT[:, g, :], in_=t_ps[:, :])
                    else:
                        nc.vector.tensor_copy(out=hT[:, g, :], in_=t_ps[:, :])
                for k in range(4):
                    if k % 2 == 0:
                        nc.vector.tensor_scalar_mul(
                            out=S_all[:, k, gs, :].rearrange("p g q -> p (g q)"),
                            in0=hT[:, gs, :].rearrange("p g q -> p (g q)"),
                            scalar1=attn_sb[:, k : k + 1],
                        )
                    else:
                        nc.scalar.activation(
                            out=S_all[:, k, gs, :].rearrange("p g q -> p (g q)"),
                            in_=hT[:, gs, :].rearrange("p g q -> p (g q)"),
                            func=AF.Copy,
                            scale=attn_sb[:, k : k + 1],
                        )

            def emit_emm_select(c):
                """e matmuls + edge-type select + exp for chunk c (xT only)."""
                g0, ch = CHUNKS[c]
                gs = slice(g0, g0 + ch)
                tg = f"s{ch}"
                alT = chnk.tile([128, ch, N], BF16, tag=f"alT{tg}")
                xe3 = chnk.tile([128, ch, N], BF16, tag=f"xe3{tg}")
                xe4 = chnk.tile([128, ch, N], BF16, tag=f"xe4{tg}")
                xT = chnk.tile([128, ch, N], BF16, tag=f"xT{tg}")
                e_c = chnk.tile([128, 4, ch, N], BF16, tag=f"e{tg}")
                chunk_state[c] = xT

                nc.vector.memset(alT[:, :, :], NEG_BIG)

                # e matmuls (rhs streams (k, j) column order)
                for gl in range(ch):
                    g = g0 + gl
                    e_ps = ps_e.tile([128, 4, N], FP32, tag="e_ps")
                    for u in range(2):
                        nc.tensor.matmul(
                            out=e_ps[u * 64 : (u + 1) * 64, :, :],
                            lhsT=hT[:, g, u * 64 : (u + 1) * 64],
                            rhs=S_all[:, :, g, u * 64 : (u + 1) * 64],
                            start=True,
                            stop=True,
                        )
                    if gl % 2 == 0:
                        nc.scalar.copy(out=e_c[:, :, gl, :], in_=e_ps[:, :, :])
                    else:
                        nc.vector.tensor_copy(out=e_c[:, :, gl, :], in_=e_ps[:, :, :])

                # transposed select over the NEG background
                # (e_k symmetric => same e bytes serve the [(v,j), g, i] view)
                for k in range(4):
                    nc.vector.copy_predicated(
                        out=alT[:, :, :],
                        mask=mt_sb[:, k, gs, :],
                        data=e_c[:, k, :, :],
                    )

                # xT = exp(leakyrelu(.)) = max(exp(.), exp(0.2 .)); NEG
                # entries give exact 0 through exp
                nc.scalar.activation(
                    out=xe3[:, :, :].rearrange(flat),
                    in_=alT[:, :, :].rearrange(flat),
                    func=AF.Exp,
                )
                nc.scalar.activation(
                    out=xe4[:, :, :].rearrange(flat),
                    in_=alT[:, :, :].rearrange(flat),
                    func=AF.Exp,
                    scale=ALPHA,
                )
                nc.vector.tensor_tensor(
                    out=xT[:, :, :], in0=xe3[:, :, :], in1=xe4[:, :, :], op=OP.max
                )

            def emit_out(c):
                """out matmuls (ones column -> row sums) + scaled evac + DMA."""
                xT = chunk_state.pop(c)
                g0, ch = CHUNKS[c]
                for gl in range(ch):
                    g = g0 + gl
                    o_ps = ps_o.tile([128, D + 1], FP32, tag="o_ps")
                    for u in range(2):
                        nc.tensor.matmul(
                            out=o_ps[u * 64 : (u + 1) * 64, :],
                            lhsT=xT[u * 64 : (u + 1) * 64, gl, :],
                            rhs=hp[u * 64 : (u + 1) * 64, g, :],
                            start=True,
                            stop=True,
                        )
                    rinv1 = outp.tile([128, 1], FP32, tag="rinv1")
                    nc.vector.reciprocal(out=rinv1[:, :], in_=o_ps[:, D : D + 1])
                    o_sb = outp.tile([128, D], FP32, tag="o_sb")
                    nc.scalar.activation(
                        out=o_sb[:, :],
                        in_=o_ps[:, 0:D],
                        func=AF.Copy,
                        scale=rinv1[:, :],
                    )
                    nc.sync.dma_start(
                        out=out_d.ap().rearrange("b i d -> (b i) d")[
                            128 * g : 128 * (g + 1), :
                        ],
                        in_=o_sb[:, :],
                    )

            # Software pipeline: ingest(c) -> e/select(c), with out(c-1)
            # interleaved so the PE has ready work while chunk c's exps run.
            emit_ingest(0)
            for c in range(NCH):
                if c + 1 < NCH:
                    emit_ingest(c + 1)
                emit_emm_select(c)
                if c >= 1:
                    emit_out(c - 1)
            emit_out(NCH - 1)
    nc.compile()
    return nc


_CACHE = {}


def _compiled():
    if "nc" not in _CACHE:
        _CACHE["nc"] = build_nc()
    return _CACHE["nc"]


def _shard_inputs(inputs, adj, embedding, attn_a):
    inputs = np.asarray(inputs)
    adj = np.asarray(adj)
    emb16 = np.ascontiguousarray(np.asarray(embedding, dtype=np.float32).astype(BF))
    attnT = np.ascontiguousarray(np.asarray(attn_a, dtype=np.float32).T)  # [D, 4]
    ident = np.ascontiguousarray(np.eye(128).astype(BF))
    in_maps = []
    for c in range(NCORES):
        sl = slice(c * BC, (c + 1) * BC)
        # idx[(u,i), g] = inputs[c*BC + 2g+u, i]
        idx = np.ascontiguousarray(
            inputs[sl].reshape(NPAIR, 2, N).transpose(1, 2, 0).reshape(128, NPAIR)
            .astype(np.int32)
        )
        adj_r = adj[sl].reshape(NPAIR, 2, N, N).astype(np.int32)  # [g, u, i, j]
        Bm = adj_r.transpose(1, 3, 0, 2).reshape(128, NPAIR, N)  # [(v,j), g, i]
        # transposed one-hot edge-type masks [p, k, g, i] uint8
        mt = np.ascontiguousarray(
            np.stack([Bm == k + 1 for k in range(4)], axis=1)
            .astype(np.uint8).reshape(128, 4 * NPAIR * N)
        )
        in_maps.append(dict(emb=emb16, idx=idx, ident=ident, mt=mt, attnT=attnT))
    return in_maps


def kernel(inputs, adj, mask_item, item, embedding, attn_a):
    in_maps = _shard_inputs(inputs, adj, embedding, attn_a)
    res = run_bass_kernel_spmd(
        _compiled(), in_maps, core_ids=list(range(NCORES))
    ).results
    out = np.concatenate([np.asarray(res[c]["out"]) for c in range(NCORES)], axis=0)
    return out.astype(np.float32)


# revision 13
# speedup vs baseline: 2.1340x; 1.0000x over previous
"""Trainium2 Bass kernel for GCE-GNN LocalAggregator (gnn_message_passing).

Computes, for each batch b:
    h = embedding[inputs]                            # [N, D] gather
    e_k = leakyrelu((h * a_k) @ h.T, 0.2)            # k = 0..3
    alpha = softmax(where(adj == k+1, e_k, -inf))    # edge-type select
    out = alpha @ h

Sharding: data-parallel over batch B=512 across 8 cores (64 batches/core).
The embedding table is staged in bf16 (11MB) per core; only looked-up rows
are read from HBM.

Key structural ideas vs the naive mapping:
  * One indirect DMA per PAIR gathers 128 rows (one per partition) into the
    pair layout h_pair[(u,i), g, d] - 32 gathers instead of 64, bf16.
  * hT (d on partitions) comes from 32 PE transposes of whole pairs.
  * e_k is SYMMETRIC in (i,j) (e_k[i,j] = sum_d h_i h_j a_k), so ONLY the
    transposed attention matrix xT is ever materialized, selected straight
    from the e buffer with host-transposed one-hot masks ([(v,j), g, i]
    reading of the same bytes). No transpose of x, no untransposed x at all.
  * h_pair is padded with a ones column (the per-pair gather leaves the
    129-col layout contiguous per instruction), so the out-matmul's last
    column yields the softmax row sums for free - the whole untransposed
    selection/exp pipeline and its masks are gone.
  * Edge-type selection is multiply-by-one-hot + reduce over k (k is the
    INNERMOST e dimension via the matmul rhs AP dim order); no-edge entries
    are zeroed exactly by a post-exp edge-mask multiply.
  * exp(leakyrelu(x)) == max(exp(x), exp(0.2x)).
  * bf16 everywhere off-PSUM; chunks of 8 pairs pipeline across engines,
    with program order interleaved so the in-order PE queue never parks
    later-chunk transposes in front of ready e-matmuls.
"""

import os
import sys

import numpy as np

for _p in ("/opt/trn_rl_repo",):
    if _p not in sys.path and os.path.isdir(_p):
        sys.path.insert(0, _p)

import ml_dtypes

import concourse.bass as bass
import concourse.bacc as bacc
import concourse.tile as tile
from concourse import mybir
from concourse.bass_utils import run_bass_kernel_spmd

B, N, D, V = 512, 64, 128, 43098
NCORES = 8
BC = B // NCORES          # 64 local batches per core
NPAIR = BC // 2           # 32 pairs
ALPHA = 0.2
NEG_BIG = -1.0e9          # exp(NEG_BIG) == 0; stands in for -9e15
CHUNKS = [(0, 8), (8, 8), (16, 8), (24, 4), (28, 4)]  # (start pair, n pairs)
NCH = len(CHUNKS)

FP32 = mybir.dt.float32
BF16 = mybir.dt.bfloat16
I32 = mybir.dt.int32
AF = mybir.ActivationFunctionType
OP = mybir.AluOpType
AX = mybir.AxisListType

BF = ml_dtypes.bfloat16


def build_nc():
    nc = bacc.Bacc("TRN2", target_bir_lowering=False, debug=False)

    emb_d = nc.dram_tensor("emb", [V, D], BF16, kind="ExternalInput")
    idx_d = nc.dram_tensor("idx", [128, NPAIR], I32, kind="ExternalInput")
    ident_d = nc.dram_tensor("ident", [128, 128], BF16, kind="ExternalInput")
    mt_d = nc.dram_tensor("mt", [128, 4 * NPAIR * N], mybir.dt.uint8, kind="ExternalInput")
    attn_d = nc.dram_tensor("attnT", [D, 4], FP32, kind="ExternalInput")
    out_d = nc.dram_tensor("out", [BC, N, D], FP32, kind="ExternalOutput")

    with tile.TileContext(nc) as tc:
        with (
            tc.tile_pool(name="singles", bufs=1) as singles,
            tc.tile_pool(name="big", bufs=1) as big,
            tc.tile_pool(name="chnk", bufs=3) as chnk,
            tc.tile_pool(name="outp", bufs=4) as outp,
            tc.tile_pool(name="ps_t", bufs=2, space="PSUM") as ps_t,
            tc.tile_pool(name="ps_e", bufs=3, space="PSUM") as ps_e,
            tc.tile_pool(name="ps_o", bufs=3, space="PSUM") as ps_o,
        ):
            # ---- tiny inputs (idx first: the gathers gate on it) ----
            idx_sb = singles.tile([128, NPAIR], I32)
            nc.sync.dma_start(out=idx_sb[:, :], in_=idx_d[:, :])
            attn_sb = singles.tile([128, 4], FP32)
            nc.sync.dma_start(out=attn_sb[:, :], in_=attn_d[:, :])
            ident = singles.tile([128, 128], BF16)
            nc.sync.dma_start(out=ident[:, :], in_=ident_d[:, :])

            # transposed edge-type masks [p, k, g, i] uint8
            mt_sb = big.tile([128, 4, NPAIR, N], mybir.dt.uint8, tag="mt")
            nc.sync.dma_start(
                out=mt_sb[:, :, :, :],
                in_=mt_d.ap().rearrange("p (k g j) -> p k g j", k=4, j=N),
            )

            # h_pair[(u,i), g, d | 1], hT[d, g, (u,i)], S[d, k, g, (u,i)]
            hp = big.tile([128, NPAIR, D + 1], BF16, tag="hp")
            hT = big.tile([128, NPAIR, 128], BF16, tag="hT")
            S_all = big.tile([128, 4, NPAIR, 128], BF16, tag="S")

            # ones column for the row-sum trick
            nc.vector.memset(hp[:, :, D : D + 1], 1.0)

            flat = "p g j -> p (g j)"
            chunk_state = {}

            def emit_ingest(c):
                """Gather + transpose + S for pair-group c."""
                g0, ch = CHUNKS[c]
                gs = slice(g0, g0 + ch)
                for g in range(g0, g0 + ch):
                    nc.gpsimd.indirect_dma_start(
                        out=hp[:, g, 0:D],
                        out_offset=None,
                        in_=emb_d[:, :],
                        in_offset=bass.IndirectOffsetOnAxis(
                            ap=idx_sb[:, g : g + 1], axis=0
                        ),
                    )
                for g in range(g0, g0 + ch):
                    t_ps = ps_t.tile([128, 128], BF16, tag="t_ps")
                    nc.tensor.transpose(
                        out=t_ps[:, :], in_=hp[:, g, 0:D], identity=ident[:, :]
                    )
                    if g % 2 == 0:
                        nc.scalar.copy(out=hT[:, g, :], in_=t_ps[:, :])
                    else:
                        nc.vector.tensor_copy(out=hT[:, g, :], in_=t_ps[:, :])
                for k in range(4):
                    if k % 2 == 0:
                        nc.vector.tensor_scalar_mul(
                            out=S_all[:, k, gs, :].rearrange("p g q -> p (g q)"),
                            in0=hT[:, gs, :].rearrange("p g q -> p (g q)"),
                            scalar1=attn_sb[:, k : k + 1],
                        )
                    else:
                        nc.scalar.activation(
                            out=S_all[:, k, gs, :].rearrange("p g q -> p (g q)"),
                            in_=hT[:, gs, :].rearrange("p g q -> p (g q)"),
                            func=AF.Copy,
                            scale=attn_sb[:, k : k + 1],
                        )

            def emit_emm_select(c):
                """e matmuls + edge-type select + exp for chunk c (xT only)."""
                g0, ch = CHUNKS[c]
                gs = slice(g0, g0 + ch)
                tg = f"s{ch}"
                alT = chnk.tile([128, ch, N], BF16, tag=f"alT{tg}")
                xe3 = chnk.tile([128, ch, N], BF16, tag=f"xe3{tg}")
                xe4 = chnk.tile([128, ch, N], BF16, tag=f"xe4{tg}")
                xT = chnk.tile([128, ch, N], BF16, tag=f"xT{tg}")
                e_c = chnk.tile([128, 4, ch, N], BF16, tag=f"e{tg}")
                chunk_state[c] = xT

                nc.vector.memset(alT[:, :, :], NEG_BIG)

                # e matmuls (rhs streams (k, j) column order)
                for gl in range(ch):
                    g = g0 + gl
                    e_ps = ps_e.tile([128, 4, N], FP32, tag="e_ps")
                    for u in range(2):
                        nc.tensor.matmul(
                            out=e_ps[u * 64 : (u + 1) * 64, :, :],
                            lhsT=hT[:, g, u * 64 : (u + 1) * 64],
                            rhs=S_all[:, :, g, u * 64 : (u + 1) * 64],
                            start=True,
                            stop=True,
                        )
                    if gl % 2 == 0:
                        nc.scalar.copy(out=e_c[:, :, gl, :], in_=e_ps[:, :, :])
                    else:
                        nc.vector.tensor_copy(out=e_c[:, :, gl, :], in_=e_ps[:, :, :])

                # transposed select over the NEG background
                # (e_k symmetric => same e bytes serve the [(v,j), g, i] view)
                for k in range(4):
                    nc.vector.copy_predicated(
                        out=alT[:, :, :],
                        mask=mt_sb[:, k, gs, :],
                        data=e_c[:, k, :, :],
                    )

                # xT = exp(leakyrelu(.)) = max(exp(.), exp(0.2 .)); NEG
                # entries give exact 0 through exp
                nc.scalar.activation(
                    out=xe3[:, :, :].rearrange(flat),
                    in_=alT[:, :, :].rearrange(flat),
                    func=AF.Exp,
                )
                nc.scalar.activation(
                    out=xe4[:, :, :].rearrange(flat),
                    in_=alT[:, :, :].rearrange(flat),
                    func=AF.Exp,
                    scale=ALPHA,
                )
                nc.vector.tensor_tensor(
                    out=xT[:, :, :], in0=xe3[:, :, :], in1=xe4[:, :, :], op=OP.max
                )

            def emit_out(c):
                """out matmuls (ones column -> row sums) + scaled evac + DMA."""
                xT = chunk_state.pop(c)
                g0, ch = CHUNKS[c]
                for gl in range(ch):
                    g = g0 + gl
                    o_ps = ps_o.tile([128, D + 1], FP32, tag="o_ps")
                    for u in range(2):
                        nc.tensor.matmul(
                            out=o_ps[u * 64 : (u + 1) * 64, :],
                            lhsT=xT[u * 64 : (u + 1) * 64, gl, :],
                            rhs=hp[u * 64 : (u + 1) * 64, g, :],
                            start=True,
                            stop=True,
                        )
                    rinv1 = outp.tile([128, 1], FP32, tag="rinv1")
                    nc.vector.reciprocal(out=rinv1[:, :], in_=o_ps[:, D : D + 1])
                    o_sb = outp.tile([128, D], FP32, tag="o_sb")
                    nc.scalar.activation(
                        out=o_sb[:, :],
                        in_=o_ps[:, 0:D],
                        func=AF.Copy,
                        scale=rinv1[:, :],
                    )
                    nc.sync.dma_start(
                        out=out_d.ap().rearrange("b i d -> (b i) d")[
                            128 * g : 128 * (g + 1), :
                        ],
                        in_=o_sb[:, :],
                    )

            # Software pipeline. PE program order per group c is
            #   T(c), e-mm(c), out-mm(c-1)
            # so the in-order PE queue never parks a later group's
            # gather-gated transposes in front of ready e-matmuls, and
            # out(c-1)'s xT is ready by the time e-mm(c) finishes.
            for c in range(NCH):
                emit_ingest(c)
                emit_emm_select(c)
                if c >= 1:
                    emit_out(c - 1)
            emit_out(NCH - 1)
    nc.compile()
    return nc


_CACHE = {}


def _compiled():
    if "nc" not in _CACHE:
        _CACHE["nc"] = build_nc()
    return _CACHE["nc"]


def _shard_inputs(inputs, adj, embedding, attn_a):
    inputs = np.asarray(inputs)
    adj = np.asarray(adj)
    emb16 = np.ascontiguousarray(np.asarray(embedding, dtype=np.float32).astype(BF))
    attnT = np.ascontiguousarray(np.asarray(attn_a, dtype=np.float32).T)  # [D, 4]
    ident = np.ascontiguousarray(np.eye(128).astype(BF))
    in_maps = []
    for c in range(NCORES):
        sl = slice(c * BC, (c + 1) * BC)
        # idx[(u,i), g] = inputs[c*BC + 2g+u, i]
        idx = np.ascontiguousarray(
            inputs[sl].reshape(NPAIR, 2, N).transpose(1, 2, 0).reshape(128, NPAIR)
            .astype(np.int32)
        )
        adj_r = adj[sl].reshape(NPAIR, 2, N, N).astype(np.int32)  # [g, u, i, j]
        Bm = adj_r.transpose(1, 3, 0, 2).reshape(128, NPAIR, N)  # [(v,j), g, i]
        # transposed one-hot edge-type masks [p, k, g, i] uint8
        mt = np.ascontiguousarray(
            np.stack([Bm == k + 1 for k in range(4)], axis=1)
            .astype(np.uint8).reshape(128, 4 * NPAIR * N)
        )
        in_maps.append(dict(emb=emb16, idx=idx, ident=ident, mt=mt, attnT=attnT))
    return in_maps


def kernel(inputs, adj, mask_item, item, embedding, attn_a):
    in_maps = _shard_inputs(inputs, adj, embedding, attn_a)
    res = run_bass_kernel_spmd(
        _compiled(), in_maps, core_ids=list(range(NCORES))
    ).results
    out = np.concatenate([np.asarray(res[c]["out"]) for c in range(NCORES)], axis=0)
    return out.astype(np.float32)


# revision 14
# speedup vs baseline: 2.1635x; 1.0139x over previous
"""Trainium2 Bass kernel for GCE-GNN LocalAggregator (gnn_message_passing).

Computes, for each batch b:
    h = embedding[inputs]                            # [N, D] gather
    e_k = leakyrelu((h * a_k) @ h.T, 0.2)            # k = 0..3
    alpha = softmax(where(adj == k+1, e_k, -inf))    # edge-type select
    out = alpha @ h

Sharding: data-parallel over batch B=512 across 8 cores (64 batches/core).
The embedding table is staged in bf16 (11MB) per core; only looked-up rows
are read from HBM.

Key structural ideas vs the naive mapping:
  * One indirect DMA per PAIR gathers 128 rows (one per partition) into the
    pair layout h_pair[(u,i), g, d] - 32 gathers instead of 64, bf16.
  * hT (d on partitions) comes from 32 PE transposes of whole pairs.
  * e_k is SYMMETRIC in (i,j) (e_k[i,j] = sum_d h_i h_j a_k), so ONLY the
    transposed attention matrix xT is ever materialized, selected straight
    from the e buffer with host-transposed one-hot masks ([(v,j), g, i]
    reading of the same bytes). No transpose of x, no untransposed x at all.
  * h_pair is padded with a ones column (the per-pair gather leaves the
    129-col layout contiguous per instruction), so the out-matmul's last
    column yields the softmax row sums for free - the whole untransposed
    selection/exp pipeline and its masks are gone.
  * Edge-type selection is multiply-by-one-hot + reduce over k (k is the
    INNERMOST e dimension via the matmul rhs AP dim order); no-edge entries
    are zeroed exactly by a post-exp edge-mask multiply.
  * exp(leakyrelu(x)) == max(exp(x), exp(0.2x)).
  * bf16 everywhere off-PSUM; chunks of 8 pairs pipeline across engines,
    with program order interleaved so the in-order PE queue never parks
    later-chunk transposes in front of ready e-matmuls.
"""

import os
import sys

import numpy as np

for _p in ("/opt/trn_rl_repo",):
    if _p not in sys.path and os.path.isdir(_p):
        sys.path.insert(0, _p)

import ml_dtypes

import concourse.bass as bass
import concourse.bacc as bacc
import concourse.tile as tile
from concourse import mybir
from concourse.bass_utils import run_bass_kernel_spmd

B, N, D, V = 512, 64, 128, 43098
NCORES = 8
BC = B // NCORES          # 64 local batches per core
NPAIR = BC // 2           # 32 pairs
ALPHA = 0.2
NEG_BIG = -1.0e9          # exp(NEG_BIG) == 0; stands in for -9e15
CHUNKS = [(0, 8), (8, 8), (16, 8), (24, 6), (30, 2)]  # (start pair, n pairs)
NCH = len(CHUNKS)

FP32 = mybir.dt.float32
BF16 = mybir.dt.bfloat16
I32 = mybir.dt.int32
AF = mybir.ActivationFunctionType
OP = mybir.AluOpType
AX = mybir.AxisListType

BF = ml_dtypes.bfloat16


def build_nc():
    nc = bacc.Bacc("TRN2", target_bir_lowering=False, debug=False)

    emb_d = nc.dram_tensor("emb", [V, D], BF16, kind="ExternalInput")
    idx_d = nc.dram_tensor("idx", [128, NPAIR], I32, kind="ExternalInput")
    ident_d = nc.dram_tensor("ident", [128, 128], BF16, kind="ExternalInput")
    mt_d = nc.dram_tensor("mt", [128, 4 * NPAIR * N], mybir.dt.uint8, kind="ExternalInput")
    attn_d = nc.dram_tensor("attnT", [D, 4], FP32, kind="ExternalInput")
    out_d = nc.dram_tensor("out", [BC, N, D], FP32, kind="ExternalOutput")

    with tile.TileContext(nc) as tc:
        with (
            tc.tile_pool(name="singles", bufs=1) as singles,
            tc.tile_pool(name="big", bufs=1) as big,
            tc.tile_pool(name="chnk", bufs=3) as chnk,
            tc.tile_pool(name="outp", bufs=4) as outp,
            tc.tile_pool(name="ps_t", bufs=2, space="PSUM") as ps_t,
            tc.tile_pool(name="ps_e", bufs=3, space="PSUM") as ps_e,
            tc.tile_pool(name="ps_o", bufs=3, space="PSUM") as ps_o,
        ):
            # ---- tiny inputs (idx first: the gathers gate on it) ----
            idx_sb = singles.tile([128, NPAIR], I32)
            nc.sync.dma_start(out=idx_sb[:, :], in_=idx_d[:, :])
            attn_sb = singles.tile([128, 4], FP32)
            nc.sync.dma_start(out=attn_sb[:, :], in_=attn_d[:, :])
            ident = singles.tile([128, 128], BF16)
            nc.sync.dma_start(out=ident[:, :], in_=ident_d[:, :])

            # transposed edge-type masks [p, k, g, i] uint8
            mt_sb = big.tile([128, 4, NPAIR, N], mybir.dt.uint8, tag="mt")
            nc.sync.dma_start(
                out=mt_sb[:, :, :, :],
                in_=mt_d.ap().rearrange("p (k g j) -> p k g j", k=4, j=N),
            )

            # h_pair[(u,i), g, d | 1], hT[d, g, (u,i)], S[d, k, g, (u,i)]
            hp = big.tile([128, NPAIR, D + 1], BF16, tag="hp")
            hT = big.tile([128, NPAIR, 128], BF16, tag="hT")
            S_all = big.tile([128, 4, NPAIR, 128], BF16, tag="S")

            # ones column for the row-sum trick
            nc.vector.memset(hp[:, :, D : D + 1], 1.0)

            flat = "p g j -> p (g j)"
            chunk_state = {}

            def emit_ingest(c):
                """Gather + transpose + S for pair-group c."""
                g0, ch = CHUNKS[c]
                gs = slice(g0, g0 + ch)
                for g in range(g0, g0 + ch):
                    nc.gpsimd.indirect_dma_start(
                        out=hp[:, g, 0:D],
                        out_offset=None,
                        in_=emb_d[:, :],
                        in_offset=bass.IndirectOffsetOnAxis(
                            ap=idx_sb[:, g : g + 1], axis=0
                        ),
                    )
                for g in range(g0, g0 + ch):
                    t_ps = ps_t.tile([128, 128], BF16, tag="t_ps")
                    nc.tensor.transpose(
                        out=t_ps[:, :], in_=hp[:, g, 0:D], identity=ident[:, :]
                    )
                    if g % 2 == 0:
                        nc.scalar.copy(out=hT[:, g, :], in_=t_ps[:, :])
                    else:
                        nc.vector.tensor_copy(out=hT[:, g, :], in_=t_ps[:, :])
                for k in range(4):
                    if k % 2 == 0:
                        nc.vector.tensor_scalar_mul(
                            out=S_all[:, k, gs, :].rearrange("p g q -> p (g q)"),
                            in0=hT[:, gs, :].rearrange("p g q -> p (g q)"),
                            scalar1=attn_sb[:, k : k + 1],
                        )
                    else:
                        nc.scalar.activation(
                            out=S_all[:, k, gs, :].rearrange("p g q -> p (g q)"),
                            in_=hT[:, gs, :].rearrange("p g q -> p (g q)"),
                            func=AF.Copy,
                            scale=attn_sb[:, k : k + 1],
                        )

            def emit_emm_select(c):
                """e matmuls + edge-type select + exp for chunk c (xT only)."""
                g0, ch = CHUNKS[c]
                gs = slice(g0, g0 + ch)
                tg = f"s{ch}"
                alT = chnk.tile([128, ch, N], BF16, tag=f"alT{tg}")
                xe3 = chnk.tile([128, ch, N], BF16, tag=f"xe3{tg}")
                xe4 = chnk.tile([128, ch, N], BF16, tag=f"xe4{tg}")
                xT = chnk.tile([128, ch, N], BF16, tag=f"xT{tg}")
                e_c = chnk.tile([128, 4, ch, N], BF16, tag=f"e{tg}")
                chunk_state[c] = xT

                nc.vector.memset(alT[:, :, :], NEG_BIG)

                # e matmuls (rhs streams (k, j) column order)
                for gl in range(ch):
                    g = g0 + gl
                    e_ps = ps_e.tile([128, 4, N], FP32, tag="e_ps")
                    for u in range(2):
                        nc.tensor.matmul(
                            out=e_ps[u * 64 : (u + 1) * 64, :, :],
                            lhsT=hT[:, g, u * 64 : (u + 1) * 64],
                            rhs=S_all[:, :, g, u * 64 : (u + 1) * 64],
                            start=True,
                            stop=True,
                        )
                    if gl % 2 == 0:
                        nc.scalar.copy(out=e_c[:, :, gl, :], in_=e_ps[:, :, :])
                    else:
                        nc.vector.tensor_copy(out=e_c[:, :, gl, :], in_=e_ps[:, :, :])

                # transposed select over the NEG background
                # (e_k symmetric => same e bytes serve the [(v,j), g, i] view)
                for k in range(4):
                    nc.vector.copy_predicated(
                        out=alT[:, :, :],
                        mask=mt_sb[:, k, gs, :],
                        data=e_c[:, k, :, :],
                    )

                # xT = exp(leakyrelu(.)) = max(exp(.), exp(0.2 .)); NEG
                # entries give exact 0 through exp
                nc.scalar.activation(
                    out=xe3[:, :, :].rearrange(flat),
                    in_=alT[:, :, :].rearrange(flat),
                    func=AF.Exp,
                )
                nc.scalar.activation(
                    out=xe4[:, :, :].rearrange(flat),
                    in_=alT[:, :, :].rearrange(flat),
                    func=AF.Exp,
                    scale=ALPHA,
                )
                nc.vector.tensor_tensor(
                    out=xT[:, :, :], in0=xe3[:, :, :], in1=xe4[:, :, :], op=OP.max
                )

            def emit_out(c):
                """out matmuls (ones column -> row sums) + scaled evac + DMA."""
                xT = chunk_state.pop(c)
                g0, ch = CHUNKS[c]
                for gl in range(ch):
                    g = g0 + gl
                    o_ps = ps_o.tile([128, D + 1], FP32, tag="o_ps")
                    for u in range(2):
                        nc.tensor.matmul(
                            out=o_ps[u * 64 : (u + 1) * 64, :],
                            lhsT=xT[u * 64 : (u + 1) * 64, gl, :],
                            rhs=hp[u * 64 : (u + 1) * 64, g, :],
                            start=True,
                            stop=True,
                        )
                    rinv1 = outp.tile([128, 1], FP32, tag="rinv1")
                    nc.vector.reciprocal(out=rinv1[:, :], in_=o_ps[:, D : D + 1])
                    o_sb = outp.tile([128, D], FP32, tag="o_sb")
                    nc.scalar.activation(
                        out=o_sb[:, :],
                        in_=o_ps[:, 0:D],
                        func=AF.Copy,
                        scale=rinv1[:, :],
                    )
                    nc.sync.dma_start(
                        out=out_d.ap().rearrange("b i d -> (b i) d")[
                            128 * g : 128 * (g + 1), :
                        ],
                        in_=o_sb[:, :],
                    )

            # Software pipeline. PE program order per group c is
            #   T(c), e-mm(c), out-mm(c-1)
            # so the in-order PE queue never parks a later group's
            # gather-gated transposes in front of ready e-matmuls, and
            # out(c-1)'s xT is ready by the time e-mm(c) finishes.
            for c in range(NCH):
                emit_ingest(c)
                if c >= 1:
                    emit_out(c - 1)
                emit_emm_select(c)
            emit_out(NCH - 1)
    nc.compile()
    return nc


_CACHE = {}


def _compiled():
    if "nc" not in _CACHE:
        _CACHE["nc"] = build_nc()
    return _CACHE["nc"]


def _shard_inputs(inputs, adj, embedding, attn_a):
    inputs = np.asarray(inputs)
    adj = np.asarray(adj)
    emb16 = np.ascontiguousarray(np.asarray(embedding, dtype=np.float32).astype(BF))
    attnT = np.ascontiguousarray(np.asarray(attn_a, dtype=np.float32).T)  # [D, 4]
    ident = np.ascontiguousarray(np.eye(128).astype(BF))
    in_maps = []
    for c in range(NCORES):
        sl = slice(c * BC, (c + 1) * BC)
        # idx[(u,i), g] = inputs[c*BC + 2g+u, i]
        idx = np.ascontiguousarray(
            inputs[sl].reshape(NPAIR, 2, N).transpose(1, 2, 0).reshape(128, NPAIR)
            .astype(np.int32)
        )
        adj_r = adj[sl].reshape(NPAIR, 2, N, N).astype(np.int32)  # [g, u, i, j]
        Bm = adj_r.transpose(1, 3, 0, 2).reshape(128, NPAIR, N)  # [(v,j), g, i]
        # transposed one-hot edge-type masks [p, k, g, i] uint8
        mt = np.ascontiguousarray(
            np.stack([Bm == k + 1 for k in range(4)], axis=1)
            .astype(np.uint8).reshape(128, 4 * NPAIR * N)
        )
        in_maps.append(dict(emb=emb16, idx=idx, ident=ident, mt=mt, attnT=attnT))
    return in_maps


def kernel(inputs, adj, mask_item, item, embedding, attn_a):
    in_maps = _shard_inputs(inputs, adj, embedding, attn_a)
    res = run_bass_kernel_spmd(
        _compiled(), in_maps, core_ids=list(range(NCORES))
    ).results
    out = np.concatenate([np.asarray(res[c]["out"]) for c in range(NCORES)], axis=0)
    return out.astype(np.float32)


# revision 15
# speedup vs baseline: 2.1733x; 1.0045x over previous
"""Trainium2 Bass kernel for GCE-GNN LocalAggregator (gnn_message_passing).

Computes, for each batch b:
    h = embedding[inputs]                            # [N, D] gather
    e_k = leakyrelu((h * a_k) @ h.T, 0.2)            # k = 0..3
    alpha = softmax(where(adj == k+1, e_k, -inf))    # edge-type select
    out = alpha @ h

Sharding: data-parallel over batch B=512 across 8 cores (64 batches/core).
The embedding table is staged in bf16 (11MB) per core; only looked-up rows
are read from HBM.

Key structural ideas vs the naive mapping:
  * One indirect DMA per PAIR gathers 128 rows (one per partition) into the
    pair layout h_pair[(u,i), g, d] - 32 gathers instead of 64, bf16.
  * hT (d on partitions) comes from 32 PE transposes of whole pairs.
  * e_k is SYMMETRIC in (i,j) (e_k[i,j] = sum_d h_i h_j a_k), so ONLY the
    transposed attention matrix xT is ever materialized, selected straight
    from the e buffer with host-transposed one-hot masks ([(v,j), g, i]
    reading of the same bytes). No transpose of x, no untransposed x at all.
  * h_pair is padded with a ones column (the per-pair gather leaves the
    129-col layout contiguous per instruction), so the out-matmul's last
    column yields the softmax row sums for free - the whole untransposed
    selection/exp pipeline and its masks are gone.
  * Edge-type selection is multiply-by-one-hot + reduce over k (k is the
    INNERMOST e dimension via the matmul rhs AP dim order); no-edge entries
    are zeroed exactly by a post-exp edge-mask multiply.
  * exp(leakyrelu(x)) == max(exp(x), exp(0.2x)).
  * bf16 everywhere off-PSUM; chunks of 8 pairs pipeline across engines,
    with program order interleaved so the in-order PE queue never parks
    later-chunk transposes in front of ready e-matmuls.
"""

import os
import sys

import numpy as np

for _p in ("/opt/trn_rl_repo",):
    if _p not in sys.path and os.path.isdir(_p):
        sys.path.insert(0, _p)

import ml_dtypes

import concourse.bass as bass
import concourse.bacc as bacc
import concourse.tile as tile
from concourse import mybir
from concourse.bass_utils import run_bass_kernel_spmd

B, N, D, V = 512, 64, 128, 43098
NCORES = 8
BC = B // NCORES          # 64 local batches per core
NPAIR = BC // 2           # 32 pairs
ALPHA = 0.2
NEG_BIG = -1.0e9          # exp(NEG_BIG) == 0; stands in for -9e15
CHUNKS = [(0, 8), (8, 8), (16, 8), (24, 4), (28, 2), (30, 2)]  # (start pair, n pairs)
NCH = len(CHUNKS)

FP32 = mybir.dt.float32
BF16 = mybir.dt.bfloat16
I32 = mybir.dt.int32
AF = mybir.ActivationFunctionType
OP = mybir.AluOpType
AX = mybir.AxisListType

BF = ml_dtypes.bfloat16


def build_nc():
    nc = bacc.Bacc("TRN2", target_bir_lowering=False, debug=False)

    emb_d = nc.dram_tensor("emb", [V, D], BF16, kind="ExternalInput")
    idx_d = nc.dram_tensor("idx", [128, NPAIR], I32, kind="ExternalInput")
    ident_d = nc.dram_tensor("ident", [128, 128], BF16, kind="ExternalInput")
    mt_d = nc.dram_tensor("mt", [128, 4 * NPAIR * N], mybir.dt.uint8, kind="ExternalInput")
    attn_d = nc.dram_tensor("attnT", [D, 4], FP32, kind="ExternalInput")
    out_d = nc.dram_tensor("out", [BC, N, D], FP32, kind="ExternalOutput")

    with tile.TileContext(nc) as tc:
        with (
            tc.tile_pool(name="singles", bufs=1) as singles,
            tc.tile_pool(name="big", bufs=1) as big,
            tc.tile_pool(name="chnk", bufs=3) as chnk,
            tc.tile_pool(name="outp", bufs=4) as outp,
            tc.tile_pool(name="ps_t", bufs=2, space="PSUM") as ps_t,
            tc.tile_pool(name="ps_e", bufs=3, space="PSUM") as ps_e,
            tc.tile_pool(name="ps_o", bufs=3, space="PSUM") as ps_o,
        ):
            # ---- tiny inputs (idx first: the gathers gate on it) ----
            idx_sb = singles.tile([128, NPAIR], I32)
            nc.sync.dma_start(out=idx_sb[:, :], in_=idx_d[:, :])
            attn_sb = singles.tile([128, 4], FP32)
            nc.sync.dma_start(out=attn_sb[:, :], in_=attn_d[:, :])
            ident = singles.tile([128, 128], BF16)
            nc.sync.dma_start(out=ident[:, :], in_=ident_d[:, :])

            # transposed edge-type masks [p, k, g, i] uint8
            mt_sb = big.tile([128, 4, NPAIR, N], mybir.dt.uint8, tag="mt")
            nc.sync.dma_start(
                out=mt_sb[:, :, :, :],
                in_=mt_d.ap().rearrange("p (k g j) -> p k g j", k=4, j=N),
            )

            # h_pair[(u,i), g, d | 1], hT[d, g, (u,i)], S[d, k, g, (u,i)]
            hp = big.tile([128, NPAIR, D + 1], BF16, tag="hp")
            hT = big.tile([128, NPAIR, 128], BF16, tag="hT")
            S_all = big.tile([128, 4, NPAIR, 128], BF16, tag="S")

            # ones column for the row-sum trick
            nc.vector.memset(hp[:, :, D : D + 1], 1.0)

            flat = "p g j -> p (g j)"
            chunk_state = {}

            def emit_ingest(c):
                """Gather + transpose + S for pair-group c."""
                g0, ch = CHUNKS[c]
                gs = slice(g0, g0 + ch)
                for g in range(g0, g0 + ch):
                    nc.gpsimd.indirect_dma_start(
                        out=hp[:, g, 0:D],
                        out_offset=None,
                        in_=emb_d[:, :],
                        in_offset=bass.IndirectOffsetOnAxis(
                            ap=idx_sb[:, g : g + 1], axis=0
                        ),
                    )
                for g in range(g0, g0 + ch):
                    t_ps = ps_t.tile([128, 128], BF16, tag="t_ps")
                    nc.tensor.transpose(
                        out=t_ps[:, :], in_=hp[:, g, 0:D], identity=ident[:, :]
                    )
                    if g % 2 == 0:
                        nc.scalar.copy(out=hT[:, g, :], in_=t_ps[:, :])
                    else:
                        nc.vector.tensor_copy(out=hT[:, g, :], in_=t_ps[:, :])
                for k in range(4):
                    if k % 2 == 0:
                        nc.vector.tensor_scalar_mul(
                            out=S_all[:, k, gs, :].rearrange("p g q -> p (g q)"),
                            in0=hT[:, gs, :].rearrange("p g q -> p (g q)"),
                            scalar1=attn_sb[:, k : k + 1],
                        )
                    else:
                        nc.scalar.activation(
                            out=S_all[:, k, gs, :].rearrange("p g q -> p (g q)"),
                            in_=hT[:, gs, :].rearrange("p g q -> p (g q)"),
                            func=AF.Copy,
                            scale=attn_sb[:, k : k + 1],
                        )

            def emit_emm_select(c):
                """e matmuls + edge-type select + exp for chunk c (xT only)."""
                g0, ch = CHUNKS[c]
                gs = slice(g0, g0 + ch)
                tg = f"s{ch}"
                alT = chnk.tile([128, ch, N], BF16, tag=f"alT{tg}")
                xe3 = chnk.tile([128, ch, N], BF16, tag=f"xe3{tg}")
                xe4 = chnk.tile([128, ch, N], BF16, tag=f"xe4{tg}")
                xT = chnk.tile([128, ch, N], BF16, tag=f"xT{tg}")
                e_c = chnk.tile([128, 4, ch, N], BF16, tag=f"e{tg}")
                chunk_state[c] = xT

                nc.vector.memset(alT[:, :, :], NEG_BIG)

                # e matmuls (rhs streams (k, j) column order)
                for gl in range(ch):
                    g = g0 + gl
                    e_ps = ps_e.tile([128, 4, N], FP32, tag="e_ps")
                    for u in range(2):
                        nc.tensor.matmul(
                            out=e_ps[u * 64 : (u + 1) * 64, :, :],
                            lhsT=hT[:, g, u * 64 : (u + 1) * 64],
                            rhs=S_all[:, :, g, u * 64 : (u + 1) * 64],
                            start=True,
                            stop=True,
                        )
                    if gl % 2 == 0:
                        nc.scalar.copy(out=e_c[:, :, gl, :], in_=e_ps[:, :, :])
                    else:
                        nc.vector.tensor_copy(out=e_c[:, :, gl, :], in_=e_ps[:, :, :])

                # transposed select over the NEG background
                # (e_k symmetric => same e bytes serve the [(v,j), g, i] view)
                for k in range(4):
                    nc.vector.copy_predicated(
                        out=alT[:, :, :],
                        mask=mt_sb[:, k, gs, :],
                        data=e_c[:, k, :, :],
                    )

                # xT = exp(leakyrelu(.)) = max(exp(.), exp(0.2 .)); NEG
                # entries give exact 0 through exp
                nc.scalar.activation(
                    out=xe3[:, :, :].rearrange(flat),
                    in_=alT[:, :, :].rearrange(flat),
                    func=AF.Exp,
                )
                nc.scalar.activation(
                    out=xe4[:, :, :].rearrange(flat),
                    in_=alT[:, :, :].rearrange(flat),
                    func=AF.Exp,
                    scale=ALPHA,
                )
                nc.vector.tensor_tensor(
                    out=xT[:, :, :], in0=xe3[:, :, :], in1=xe4[:, :, :], op=OP.max
                )

            def emit_out(c):
                """out matmuls (ones column -> row sums) + scaled evac + DMA."""
                xT = chunk_state.pop(c)
                g0, ch = CHUNKS[c]
                for gl in range(ch):
                    g = g0 + gl
                    o_ps = ps_o.tile([128, D + 1], FP32, tag="o_ps")
                    for u in range(2):
                        nc.tensor.matmul(
                            out=o_ps[u * 64 : (u + 1) * 64, :],
                            lhsT=xT[u * 64 : (u + 1) * 64, gl, :],
                            rhs=hp[u * 64 : (u + 1) * 64, g, :],
                            start=True,
                            stop=True,
                        )
                    rinv1 = outp.tile([128, 1], FP32, tag="rinv1")
                    nc.vector.reciprocal(out=rinv1[:, :], in_=o_ps[:, D : D + 1])
                    o_sb = outp.tile([128, D], FP32, tag="o_sb")
                    nc.scalar.activation(
                        out=o_sb[:, :],
                        in_=o_ps[:, 0:D],
                        func=AF.Copy,
                        scale=rinv1[:, :],
                    )
                    nc.sync.dma_start(
                        out=out_d.ap().rearrange("b i d -> (b i) d")[
                            128 * g : 128 * (g + 1), :
                        ],
                        in_=o_sb[:, :],
                    )

            # Software pipeline. PE program order per group c is
            #   T(c), e-mm(c), out-mm(c-1)
            # so the in-order PE queue never parks a later group's
            # gather-gated transposes in front of ready e-matmuls, and
            # out(c-1)'s xT is ready by the time e-mm(c) finishes.
            for c in range(NCH):
                emit_ingest(c)
                if c >= 1:
                    emit_out(c - 1)
                emit_emm_select(c)
            emit_out(NCH - 1)
    nc.compile()
    return nc


_CACHE = {}


def _compiled():
    if "nc" not in _CACHE:
        _CACHE["nc"] = build_nc()
    return _CACHE["nc"]


def _shard_inputs(inputs, adj, embedding, attn_a):
    inputs = np.asarray(inputs)
    adj = np.asarray(adj)
    emb16 = np.ascontiguousarray(np.asarray(embedding, dtype=np.float32).astype(BF))
    attnT = np.ascontiguousarray(np.asarray(attn_a, dtype=np.float32).T)  # [D, 4]
    ident = np.ascontiguousarray(np.eye(128).astype(BF))
    in_maps = []
    for c in range(NCORES):
        sl = slice(c * BC, (c + 1) * BC)
        # idx[(u,i), g] = inputs[c*BC + 2g+u, i]
        idx = np.ascontiguousarray(
            inputs[sl].reshape(NPAIR, 2, N).transpose(1, 2, 0).reshape(128, NPAIR)
            .astype(np.int32)
        )
        adj_r = adj[sl].reshape(NPAIR, 2, N, N).astype(np.int32)  # [g, u, i, j]
        Bm = adj_r.transpose(1, 3, 0, 2).reshape(128, NPAIR, N)  # [(v,j), g, i]
        # transposed one-hot edge-type masks [p, k, g, i] uint8
        mt = np.ascontiguousarray(
            np.stack([Bm == k + 1 for k in range(4)], axis=1)
            .astype(np.uint8).reshape(128, 4 * NPAIR * N)
        )
        in_maps.append(dict(emb=emb16, idx=idx, ident=ident, mt=mt, attnT=attnT))
    return in_maps


def kernel(inputs, adj, mask_item, item, embedding, attn_a):
    in_maps = _shard_inputs(inputs, adj, embedding, attn_a)
    res = run_bass_kernel_spmd(
        _compiled(), in_maps, core_ids=list(range(NCORES))
    ).results
    out = np.concatenate([np.asarray(res[c]["out"]) for c in range(NCORES)], axis=0)
    return out.astype(np.float32)
